# revision 1
# baseline (speedup 1.0000x reference)
"""GRU Seq2Seq Trainium2 kernel (nn_GRU_Seq2Seq_83219286327778).

Strategy: data-parallel over batch (2048 -> 8 x 256), gate-major transposed
layout on-device ([hidden/gate dim on partitions, batch on free dim]) so the
recurrence needs no transposes. gx+gh accumulate in PSUM; biases folded into
matmuls (ones-row / K=1 rank-1 matmuls); fc4 feedback folded into the next
step's gx via Wcomb = dW0 @ W4. Matmuls run in float32r (tf32-class).
"""
import sys
sys.path.insert(0, "/opt/trn_rl_repo")
import numpy as np

B, LAGS, HORIZONS, F, H = 2048, 64, 24, 64, 512
NCORES = 8
BL = B // NCORES           # 256 batch per core
G3 = 3 * H                 # 1536
KC = H // 128              # 4 k-chunks
SRC_CHUNK = 8              # timesteps per src DMA

_RUNNER = None


def _build_nc(mm_dt_name="float32r", repeat=1, lags=LAGS, horizons=HORIZONS, dump_h=False):
    import concourse.tile as tile
    from concourse import mybir, bacc

    F32 = mybir.dt.float32
    MMD = getattr(mybir.dt, mm_dt_name)
    AF = mybir.ActivationFunctionType
    OP = mybir.AluOpType

    nc = bacc.Bacc("TRN2", target_bir_lowering=False)

    srcT_d = nc.dram_tensor("srcT", [F + 1, LAGS, BL], F32, kind="ExternalInput")
    wnames = ["eu0", "ew1", "eu1", "du0", "dw1", "du1", "wcomb"]
    w_d = {n: nc.dram_tensor(n, [H, G3], F32, kind="ExternalInput") for n in wnames}
    ew0a_d = nc.dram_tensor("ew0a", [F + 1, G3], F32, kind="ExternalInput")
    dw0a_d = nc.dram_tensor("dw0a", [F + 1, G3], F32, kind="ExternalInput")
    br_d = nc.dram_tensor("biasrows", [1, 3 * G3], F32, kind="ExternalInput")
    cns_d = nc.dram_tensor("cns", [128, 16], F32, kind="ExternalInput")
    w1t_d = nc.dram_tensor("w1t", [128, KC], F32, kind="ExternalInput")
    ones_d = nc.dram_tensor("onesr", [1, BL], F32, kind="ExternalInput")
    out_d = nc.dram_tensor("out", [HORIZONS, BL], F32, kind="ExternalOutput")
    h0o_d = h1o_d = None
    if dump_h:
        h0o_d = nc.dram_tensor("h0o", [128, KC, BL], F32, kind="ExternalOutput")
        h1o_d = nc.dram_tensor("h1o", [128, KC, BL], F32, kind="ExternalOutput")

    with tile.TileContext(nc) as tc:
        with tc.tile_pool(name="wp", bufs=1) as wp, \
             tc.tile_pool(name="sp", bufs=2) as sp, \
             tc.tile_pool(name="hp", bufs=1) as hp, \
             tc.tile_pool(name="gp", bufs=2) as gp, \
             tc.tile_pool(name="op_", bufs=2) as opool, \
             tc.tile_pool(name="pp", bufs=1, space="PSUM") as pp:

            # ---- persistent small tensors ----
            br_t = wp.tile([1, 3 * G3], MMD, tag="br", name="br")
            nc.gpsimd.dma_start(br_t[:], br_d[:])
            cns_t = wp.tile([128, 16], F32, tag="cns", name="cns")
            nc.sync.dma_start(cns_t[:], cns_d[:])
            w1t_t = wp.tile([128, KC], MMD, tag="w1t", name="w1t")
            nc.gpsimd.dma_start(w1t_t[:], w1t_d[:])
            ones_t = wp.tile([1, BL], MMD, tag="ones", name="ones")
            nc.gpsimd.dma_start(ones_t[:], ones_d[:])
            ew0a_t = wp.tile([F + 1, G3], MMD, tag="w0a", name="w0a")
            nc.gpsimd.dma_start(ew0a_t[:], ew0a_d[:])
            dw0a_t = wp.tile([F + 1, G3], MMD, tag="dw0a", name="dw0a")
            nc.gpsimd.dma_start(dw0a_t[:], dw0a_d[:])

            def load_u(dram, tagbase):
                ts_ = []
                for k in range(KC):
                    t = wp.tile([128, G3], MMD, tag=f"{tagbase}{k}", name=f"{tagbase}{k}")
                    nc.gpsimd.dma_start(t[:], dram[k * 128:(k + 1) * 128, :])
                    ts_.append(t)
                return ts_

            # encoder weights (slots uA/uB/uC), wcomb has its own slot
            eu0_t = load_u(w_d["eu0"], "uA")
            ew1_t = load_u(w_d["ew1"], "uB")
            eu1_t = load_u(w_d["eu1"], "uC")
            wcomb_t = load_u(w_d["wcomb"], "uD")

            # hidden state ping-pong, float32r so matmuls can read directly
            h0b = [hp.tile([128, KC, BL], MMD, tag=f"h0{i}", name=f"h0{i}") for i in range(2)]
            h1b = [hp.tile([128, KC, BL], MMD, tag=f"h1{i}", name=f"h1{i}") for i in range(2)]

            def cell(gx_rhs, gx_lhs, gh_lhs, biasrow, cn_col, h_prev, h_out):
                """One GRU cell step in gate-major layout.

                gx_rhs: list of rhs APs (K-chunks) for the input projection
                gx_lhs: list of lhsT tiles matching gx_rhs ([*,G3] each)
                gh_lhs: 4 lhsT tiles for the recurrent projection
                biasrow: row index into br_t for bias rank-1 matmul, or None
                          (None => gx carries an ones-row that adds biases)
                cn_col: column in cns_t holding this cell's hidden n-bias (x4)
                """
                pa = [pp.tile([128, 512], F32, tag=f"pA{m}", name=f"pA{m}") for m in range(4)]
                pb = [pp.tile([128, 512], F32, tag=f"pB{m}", name=f"pB{m}") for m in range(4)]
                rz = gp.tile([128, 4, 512], F32, tag="rz", name="rz")
                tt = gp.tile([128, 4, BL], F32, tag="g1", name="tt")
                vv = gp.tile([128, 4, BL], F32, tag="g2", name="vv")
                nn = gp.tile([128, 4, BL], F32, tag="gn", name="nn")

                def gate_group(out_ap, g):
                    """accumulate gx+gh(+bias) for gate-tile g into out_ap"""
                    first = True
                    for lhs, rhs in zip(gx_lhs, gx_rhs, strict=True):
                        nc.tensor.matmul(out_ap, lhs[:, g * 128:(g + 1) * 128], rhs,
                                         start=first, stop=False)
                        first = False
                    n_gh = len(gh_lhs)
                    for k in range(n_gh):
                        last = (k == n_gh - 1) and biasrow is None
                        nc.tensor.matmul(out_ap, gh_lhs[k][:, g * 128:(g + 1) * 128],
                                         h_prev[:, k, :], start=False, stop=last)
                    if biasrow is not None:
                        nc.tensor.matmul(out_ap, br_t[0:1, biasrow * G3 + g * 128:biasrow * G3 + (g + 1) * 128],
                                         ones_t[:], start=False, stop=True)

                def gx_only_group(out_ap, g):
                    first = True
                    for lhs, rhs in zip(gx_lhs, gx_rhs, strict=True):
                        is_last = (lhs is gx_lhs[-1]) and biasrow is None
                        nc.tensor.matmul(out_ap, lhs[:, g * 128:(g + 1) * 128], rhs,
                                         start=first, stop=is_last)
                        first = False
                    if biasrow is not None:
                        nc.tensor.matmul(out_ap, br_t[0:1, biasrow * G3 + g * 128:biasrow * G3 + (g + 1) * 128],
                                         ones_t[:], start=False, stop=True)

                def gh_only_group(out_ap, g):
                    for k in range(len(gh_lhs)):
                        nc.tensor.matmul(out_ap, gh_lhs[k][:, g * 128:(g + 1) * 128],
                                         h_prev[:, k, :], start=(k == 0),
                                         stop=(k == len(gh_lhs) - 1))

                # r/z banks + sigmoid
                for m in range(4):
                    gate_group(pa[m][:, 0:BL], m)          # r
                    gate_group(pa[m][:, BL:2 * BL], 4 + m)  # z
                    nc.scalar.activation(rz[:, m, :], pa[m][:, :], AF.Sigmoid)
                # xn | hn banks
                for m in range(4):
                    gx_only_group(pb[m][:, 0:BL], 8 + m)    # xn (+ bias)
                    gh_only_group(pb[m][:, BL:2 * BL], 8 + m)  # hn (no bias)
                    # t = (hn + cn) * r
                    nc.vector.scalar_tensor_tensor(
                        tt[:, m, :], pb[m][:, BL:2 * BL], cns_t[:, cn_col * 4 + m:cn_col * 4 + m + 1],
                        rz[:, m, 0:BL], OP.add, OP.mult)
                    # v = t + xn
                    nc.vector.tensor_tensor(vv[:, m, :], tt[:, m, :], pb[m][:, 0:BL], OP.add)
                nc.scalar.activation(nn[:, :, :], vv[:, :, :], AF.Tanh)
                # h' = n + z*(h - n)
                dd = gp.tile([128, 4, BL], F32, tag="g1", name="dd")
                ee = gp.tile([128, 4, BL], F32, tag="g2", name="ee")
                nc.vector.tensor_tensor(dd[:], h_prev.bitcast(F32)[:, 0:KC, :], nn[:], OP.subtract)
                nc.vector.tensor_tensor(ee[:], rz[:, :, BL:2 * BL], dd[:], OP.mult)
                nc.vector.tensor_tensor(h_out[:, 0:KC, :], ee[:], nn[:], OP.add)

            for _rep in range(repeat):
                for i in range(2):
                    nc.vector.memzero(h0b[i][:])
                    nc.vector.memzero(h1b[i][:])

                # ---------------- encoder ----------------
                sc = None
                for t in range(lags):
                    if t % SRC_CHUNK == 0:
                        sc = sp.tile([F + 1, SRC_CHUNK, BL], MMD, tag="src", name=f"src{t}")
                        nc.gpsimd.dma_start(
                            sc[:], srcT_d[:, t:t + SRC_CHUNK, :])
                    j = t % SRC_CHUNK
                    p, q = t % 2, (t + 1) % 2
                    cell([sc[:, j, :]], [ew0a_t], eu0_t, None, 0, h0b[p], h0b[q])
                    cell([h0b[q][:, k, :] for k in range(KC)], ew1_t, eu1_t, 0, 1,
                         h1b[p], h1b[q])
                sc_last = sc
                if dump_h:
                    pfin = lags % 2
                    h0c = gp.tile([128, KC, BL], F32, tag="g1", name="h0c")
                    nc.vector.tensor_copy(h0c[:], h0b[pfin].bitcast(F32)[:, 0:KC, :])
                    nc.sync.dma_start(h0o_d[:], h0c[:])
                    h1c = gp.tile([128, KC, BL], F32, tag="g2", name="h1c")
                    nc.vector.tensor_copy(h1c[:], h1b[pfin].bitcast(F32)[:, 0:KC, :])
                    nc.sync.dma_start(h1o_d[:], h1c[:])

                # swap in decoder recurrent weights (reuse encoder slots)
                du0_t = load_u(w_d["du0"], "uA")
                dw1_t = load_u(w_d["dw1"], "uB")
                du1_t = load_u(w_d["du1"], "uC")

                # ---------------- decoder ----------------
                for d in range(horizons):
                    p, q = (lags + d) % 2, (lags + d + 1) % 2
                    if d == 0:
                        cell([sc_last[:, (lags - 1) % SRC_CHUNK, :]], [dw0a_t],
                             du0_t, None, 2, h0b[p], h0b[q])
                    else:
                        cell([h1b[p][:, k, :] for k in range(KC)], wcomb_t,
                             du0_t, 1, 2, h0b[p], h0b[q])
                    cell([h0b[q][:, k, :] for k in range(KC)], dw1_t, du1_t, 2, 3,
                         h1b[p], h1b[q])
                    # out1[d] = W1 . h1_new   (b1 added on host)
                    po = pp.tile([128, 512], F32, tag="pA0", name=f"po{d}")
                    for k in range(KC):
                        nc.tensor.matmul(po[0:1, 0:BL], w1t_t[:, k:k + 1],
                                         h1b[q][:, k, :], start=(k == 0), stop=(k == KC - 1))
                    osb = opool.tile([1, BL], F32, tag="o1", name=f"o{d}")
                    nc.scalar.copy(osb[:], po[0:1, 0:BL])
                    nc.sync.dma_start(out_d[d:d + 1, :], osb[:])

                if repeat > 1 and _rep + 1 < repeat:
                    # reload encoder weights for the next timing repetition
                    eu0_t = load_u(w_d["eu0"], "uA")
                    ew1_t = load_u(w_d["ew1"], "uB")
                    eu1_t = load_u(w_d["eu1"], "uC")

    nc.compile()
    return nc


def _host_prep(inputs):
    f32 = np.float32
    g = {k: np.asarray(v, dtype=f32) if np.asarray(v).dtype != np.int64 else v
         for k, v in inputs.items()}
    src = np.asarray(inputs["src"], f32)
    eW0, eU0, eb0, ec0 = g["eW0"], g["eU0"], g["eb0"], g["ec0"]
    eW1, eU1, eb1, ec1 = g["eW1"], g["eU1"], g["eb1"], g["ec1"]
    dW0, dU0, db0, dc0 = g["dW0"], g["dU0"], g["db0"], g["dc0"]
    dW1, dU1, db1, dc1 = g["dW1"], g["dU1"], g["db1"], g["dc1"]
    W1, b1, W4, b4 = g["W1"], g["b1"], g["W4"], g["b4"]

    def rzn_bias(b, c):
        return np.concatenate([b[0:H] + c[0:H], b[H:2 * H] + c[H:2 * H], b[2 * H:]])

    Wcomb = (dW0 @ W4).astype(f32)                       # [1536, 512]
    dcomb = (db0 + dW0 @ b4).astype(f32)                 # [1536]
    shared = {
        "eu0": eU0.T.copy(), "ew1": eW1.T.copy(), "eu1": eU1.T.copy(),
        "du0": dU0.T.copy(), "dw1": dW1.T.copy(), "du1": dU1.T.copy(),
        "wcomb": Wcomb.T.copy(),
        "ew0a": np.concatenate([eW0.T, rzn_bias(eb0, ec0)[None, :]], 0),
        "dw0a": np.concatenate([dW0.T, rzn_bias(db0, dc0)[None, :]], 0),
        "biasrows": np.concatenate([rzn_bias(eb1, ec1), rzn_bias(dcomb, dc0),
                                    rzn_bias(db1, dc1)])[None, :],
        "cns": np.stack([c[2 * H:].reshape(KC, 128).T.reshape(128, KC)
                         for c in (ec0, ec1, dc0, dc1)], 1).reshape(128, 4 * KC)
                  .astype(f32),
        "w1t": W1[0].reshape(KC, 128).T.copy(),
        "onesr": np.ones((1, BL), f32),
    }
    # cns layout fix: want cns[:, c*4+m] = c_n[m*128+p]
    cns = np.zeros((128, 16), f32)
    for ci, c in enumerate((ec0, ec1, dc0, dc1)):
        cn = c[2 * H:]
        for m in range(KC):
            cns[:, ci * 4 + m] = cn[m * 128:(m + 1) * 128]
    shared["cns"] = cns
    shared = {k: np.ascontiguousarray(v, dtype=f32) for k, v in shared.items()}

    in_maps = []
    for c in range(NCORES):
        s = src[c * BL:(c + 1) * BL]                     # [256, 64, 64]
        sT = np.ascontiguousarray(s.transpose(2, 1, 0))  # [64, 64, 256]
        sA = np.concatenate([sT, np.ones((1, LAGS, BL), f32)], 0)
        m = dict(shared)
        m["srcT"] = np.ascontiguousarray(sA)
        in_maps.append(m)
    return in_maps, float(b1[0])


class _Runner:
    """Build-once sharded PJRT runner (axon: 8 NeuronCores)."""

    def __init__(self, nc):
        import jax
        from jax.sharding import Mesh, PartitionSpec
        from jax.experimental.shard_map import shard_map
        from concourse import mybir
        from concourse.bass2jax import (_bass_exec_p, partition_id_tensor,
                                        install_neuronx_cc_hook)
        install_neuronx_cc_hook()
        self.jax = jax
        partition_name = nc.partition_id_tensor.name if nc.partition_id_tensor else None
        in_names, out_names, out_avals, zero_outs = [], [], [], []
        for alloc in nc.m.functions[0].allocations:
            if not isinstance(alloc, mybir.MemoryLocationSet):
                continue
            name = alloc.memorylocations[0].name
            if alloc.kind == "ExternalInput":
                if name != partition_name:
                    in_names.append(name)
            elif alloc.kind == "ExternalOutput":
                out_names.append(name)
                shape = tuple(alloc.tensor_shape)
                dtype = mybir.dt.np(alloc.dtype)
                out_avals.append(jax.core.ShapedArray(shape, dtype))
                zero_outs.append(np.zeros(shape, dtype))
        n_params = len(in_names)
        all_in = list(in_names) + list(out_names)
        if partition_name is not None:
            all_in.append(partition_name)
        self.in_names, self.out_names = in_names, out_names
        self.out_avals, self.zero_outs = out_avals, zero_outs

        def _body(*args):
            operands = list(args)
            if partition_name is not None:
                operands.append(partition_id_tensor())
            return tuple(_bass_exec_p.bind(
                *operands, out_avals=tuple(out_avals), in_names=tuple(all_in),
                out_names=tuple(out_names), lowering_input_output_aliases=(),
                sim_require_finite=True, sim_require_nnan=True, nc=nc))

        devices = jax.devices()[:NCORES]
        self.mesh = Mesh(np.asarray(devices), ("core",))
        in_specs = (PartitionSpec("core"),) * (n_params + len(out_names))
        out_specs = (PartitionSpec("core"),) * len(out_names)
        donate = tuple(range(n_params, n_params + len(out_names)))
        self.fn = jax.jit(
            shard_map(_body, mesh=self.mesh, in_specs=in_specs,
                      out_specs=out_specs, check_rep=False),
            donate_argnums=donate, keep_unused=True)
        self.sh = jax.sharding.NamedSharding(self.mesh, PartitionSpec("core"))

    def place(self, in_maps):
        n = NCORES
        self.placed = [
            self.jax.device_put(np.ascontiguousarray(
                np.concatenate([in_maps[c][nm] for c in range(n)], 0)), self.sh)
            for nm in self.in_names]

    def run(self):
        zeros = [self.jax.device_put(
            np.zeros((NCORES * z.shape[0], *z.shape[1:]), z.dtype), self.sh)
            for z in self.zero_outs]
        outs = self.fn(*self.placed, *zeros)
        self.jax.block_until_ready(outs)
        return outs

    def results(self, outs):
        return [
            {nm: np.asarray(outs[i]).reshape(NCORES, *self.out_avals[i].shape)[c]
             for i, nm in enumerate(self.out_names)}
            for c in range(NCORES)]


def get_runner(repeat=1):
    global _RUNNER
    key = ("r", repeat)
    if _RUNNER is None or _RUNNER[0] != key:
        nc = _build_nc(repeat=repeat)
        _RUNNER = (key, _Runner(nc))
    return _RUNNER[1]


def kernel(**inputs) -> np.ndarray:
    in_maps, b1 = _host_prep(inputs)
    r = get_runner()
    r.place(in_maps)
    res = r.results(r.run())
    out = np.empty((B, HORIZONS), np.float32)
    for c in range(NCORES):
        out[c * BL:(c + 1) * BL] = res[c]["out"].T + b1
    return out



# revision 3
# speedup vs baseline: 1.0179x; 1.0179x over previous
"""GRU Seq2Seq Trainium2 kernel (nn_GRU_Seq2Seq_83219286327778).

Strategy: data-parallel over batch (2048 -> 8 x 256), gate-major transposed
layout on-device ([hidden/gate dim on partitions, batch on free dim]) so the
recurrence needs no transposes. Matmuls in bf16 (fp32 PSUM accumulate) to make
the per-matmul LDWEIGHTS cheap enough to hide under the moving stream; all
weights SBUF-resident from the start; biases folded into activation bias APs
and fused DVE ops (no rank-1 bias matmuls); gh emitted before gx in cells
whose input comes from freshly-computed state so the PE never stalls.
fc4 feedback folded into the next step's gx via Wcomb = dW0 @ W4.
"""
import sys
sys.path.insert(0, "/opt/trn_rl_repo")
import numpy as np

B, LAGS, HORIZONS, F, H = 2048, 64, 24, 64, 512
NCORES = 8
BL = B // NCORES           # 256 batch per core
G3 = 3 * H                 # 1536
KC = H // 128              # 4 k-chunks
SRC_CHUNK = 8              # timesteps per src DMA

# bias column layout in the [128, 76] biases tensor
# rz: ct*8 + g*4 + m   (ct in 0..4, g: 0=r 1=z, m tile)    cols 0..39
# bn: 40 + ct*4 + m    (x-side n bias per celltype)        cols 40..59
# cn: 60 + c4*4 + m    (h-side n bias per U-set)           cols 60..75
CT_ENC0, CT_ENC1, CT_DEC0F, CT_DEC0, CT_DEC1 = range(5)
C4_ENC0, C4_ENC1, C4_DEC0, C4_DEC1 = range(4)

_RUNNER = None


def _build_nc(repeat=1, lags=LAGS, horizons=HORIZONS):
    import concourse.tile as tile
    from concourse import mybir, bacc

    F32 = mybir.dt.float32
    BF = mybir.dt.bfloat16
    AF = mybir.ActivationFunctionType
    OP = mybir.AluOpType

    nc = bacc.Bacc("TRN2", target_bir_lowering=False)

    srcT_d = nc.dram_tensor("srcT", [F, LAGS, BL], BF, kind="ExternalInput")
    wnames = ["eu0", "ew1", "eu1", "du0", "dw1", "du1", "wcomb"]
    w_d = {n: nc.dram_tensor(n, [H, G3], BF, kind="ExternalInput") for n in wnames}
    ew0_d = nc.dram_tensor("ew0", [F, G3], BF, kind="ExternalInput")
    dw0_d = nc.dram_tensor("dw0", [F, G3], BF, kind="ExternalInput")
    bias_d = nc.dram_tensor("biases", [128, 76], F32, kind="ExternalInput")
    w1t_d = nc.dram_tensor("w1t", [128, KC], BF, kind="ExternalInput")
    out_d = nc.dram_tensor("out", [HORIZONS, BL], F32, kind="ExternalOutput")

    with tile.TileContext(nc) as tc:
        with tc.tile_pool(name="wp", bufs=1) as wp, \
             tc.tile_pool(name="sp", bufs=2) as sp, \
             tc.tile_pool(name="hp", bufs=1) as hp, \
             tc.tile_pool(name="gp", bufs=2) as gp, \
             tc.tile_pool(name="op_", bufs=2) as opool, \
             tc.tile_pool(name="pp", bufs=1, space="PSUM") as pp:

            # ---- persistent small tensors ----
            bias_t = wp.tile([128, 76], F32, tag="bias", name="bias")
            nc.sync.dma_start(bias_t[:], bias_d[:])
            w1t_t = wp.tile([128, KC], BF, tag="w1t", name="w1t")
            nc.gpsimd.dma_start(w1t_t[:], w1t_d[:])
            ew0_t = wp.tile([F, G3], BF, tag="w0a", name="w0a")
            nc.gpsimd.dma_start(ew0_t[:], ew0_d[:])
            dw0_t = wp.tile([F, G3], BF, tag="dw0a", name="dw0a")
            nc.gpsimd.dma_start(dw0_t[:], dw0_d[:])

            def load_u(dram, tagbase):
                ts_ = []
                for k in range(KC):
                    t = wp.tile([128, G3], BF, tag=f"{tagbase}{k}", name=f"{tagbase}{k}")
                    nc.gpsimd.dma_start(t[:], dram[k * 128:(k + 1) * 128, :])
                    ts_.append(t)
                return ts_

            # all weights resident for the whole kernel
            eu0_t = load_u(w_d["eu0"], "uA")
            ew1_t = load_u(w_d["ew1"], "uB")
            eu1_t = load_u(w_d["eu1"], "uC")
            du0_t = load_u(w_d["du0"], "uD")
            dw1_t = load_u(w_d["dw1"], "uE")
            du1_t = load_u(w_d["du1"], "uF")
            wcomb_t = load_u(w_d["wcomb"], "uG")

            # hidden state ping-pong (bf16: matmul rhs + 2x DVE)
            h0b = [hp.tile([128, KC, BL], BF, tag=f"h0{i}", name=f"h0{i}") for i in range(2)]
            h1b = [hp.tile([128, KC, BL], BF, tag=f"h1{i}", name=f"h1{i}") for i in range(2)]

            def rzb(ct, g, m):
                c = ct * 8 + g * 4 + m
                return bias_t[:, c:c + 1]

            def bnb(ct, m):
                c = 40 + ct * 4 + m
                return bias_t[:, c:c + 1]

            def cnb(c4, m):
                c = 60 + c4 * 4 + m
                return bias_t[:, c:c + 1]

            def cell(gx_lhs, gx_rhs, gh_lhs, h_prev, h_out, ct, c4, gx_first):
                """One GRU cell step, gate-major.

                gx_lhs/gx_rhs: matching lists of lhsT tiles ([*, G3]) and rhs APs
                gh_lhs: KC lhsT tiles for the recurrent projection
                h_prev/h_out: [128, KC, BL] bf16 state tiles
                ct/c4: bias column groups; gx_first: emit gx phase before gh

                PSUM bank packing (one pending accumulation group per 2KB bank):
                pa[m] = r | hn, pb[m] = z | xn. The r/z groups stay open across
                the two phases; hn/xn are single-phase groups, ordered so each
                bank's groups are strictly sequential.
                """
                pa = [pp.tile([128, 512], F32, tag=f"pA{m}", name=f"pA{m}") for m in range(4)]
                pb = [pp.tile([128, 512], F32, tag=f"pB{m}", name=f"pB{m}") for m in range(4)]
                rz = gp.tile([128, 4, 512], BF, tag="rz", name="rz")
                tt = gp.tile([128, 4, BL], BF, tag="g1", name="tt")
                vv = gp.tile([128, 4, BL], BF, tag="g2", name="vv")
                nn = gp.tile([128, 4, BL], BF, tag="gn", name="nn")
                hp_chunks = [h_prev[:, k, :] for k in range(KC)]

                def emit(lhs_list, rhs_list, m, goff, out_ap, opening, closing):
                    n = len(lhs_list)
                    for i, (lhs, rhs) in enumerate(zip(lhs_list, rhs_list, strict=True)):
                        nc.tensor.matmul(
                            out_ap, lhs[:, goff + m * 128:goff + (m + 1) * 128], rhs,
                            start=(opening and i == 0), stop=(closing and i == n - 1))

                def epilogue(m):
                    # r/z/xn/hn for tile m all closed: sigmoids (+bias) + n folds
                    nc.scalar.activation(rz[:, m, 0:BL], pa[m][:, 0:BL],
                                         AF.Sigmoid, bias=rzb(ct, 0, m))
                    nc.scalar.activation(rz[:, m, BL:2 * BL], pb[m][:, 0:BL],
                                         AF.Sigmoid, bias=rzb(ct, 1, m))
                    # tt = (hn + cn) * r ; vv = (xn + bn) + tt
                    nc.vector.scalar_tensor_tensor(
                        tt[:, m, :], pa[m][:, BL:2 * BL], cnb(c4, m),
                        rz[:, m, 0:BL], OP.add, OP.mult)
                    nc.vector.scalar_tensor_tensor(
                        vv[:, m, :], pb[m][:, BL:2 * BL], bnb(ct, m),
                        tt[:, m, :], OP.add, OP.add)

                if gx_first:
                    for m in range(4):
                        emit(gx_lhs, gx_rhs, m, 1024, pb[m][:, BL:2 * BL], True, True)   # xn
                        emit(gx_lhs, gx_rhs, m, 0, pa[m][:, 0:BL], True, False)          # r open
                        emit(gx_lhs, gx_rhs, m, 512, pb[m][:, 0:BL], True, False)        # z open
                    for m in range(4):
                        emit(gh_lhs, hp_chunks, m, 0, pa[m][:, 0:BL], False, True)       # r close
                        emit(gh_lhs, hp_chunks, m, 512, pb[m][:, 0:BL], False, True)     # z close
                        emit(gh_lhs, hp_chunks, m, 1024, pa[m][:, BL:2 * BL], True, True)  # hn
                        epilogue(m)
                else:
                    for m in range(4):
                        emit(gh_lhs, hp_chunks, m, 1024, pa[m][:, BL:2 * BL], True, True)  # hn
                        emit(gh_lhs, hp_chunks, m, 0, pa[m][:, 0:BL], True, False)       # r open
                        emit(gh_lhs, hp_chunks, m, 512, pb[m][:, 0:BL], True, False)     # z open
                    for m in range(4):
                        emit(gx_lhs, gx_rhs, m, 0, pa[m][:, 0:BL], False, True)          # r close
                        emit(gx_lhs, gx_rhs, m, 512, pb[m][:, 0:BL], False, True)        # z close
                        emit(gx_lhs, gx_rhs, m, 1024, pb[m][:, BL:2 * BL], True, True)   # xn
                        epilogue(m)

                nc.scalar.activation(nn[:, :, :], vv[:, :, :], AF.Tanh)
                # h' = n + z*(h - n)
                dd = gp.tile([128, 4, BL], BF, tag="g1", name="dd")
                ee = gp.tile([128, 4, BL], BF, tag="g2", name="ee")
                nc.vector.tensor_tensor(dd[:], h_prev[:, 0:KC, :], nn[:], OP.subtract)
                nc.vector.tensor_tensor(ee[:], rz[:, :, BL:2 * BL], dd[:], OP.mult)
                nc.vector.tensor_tensor(h_out[:, 0:KC, :], ee[:], nn[:], OP.add)

            for _rep in range(repeat):
                for i in range(2):
                    nc.vector.memzero(h0b[i][:])
                    nc.vector.memzero(h1b[i][:])

                # ---------------- encoder ----------------
                sc = None
                for t in range(lags):
                    if t % SRC_CHUNK == 0:
                        sc = sp.tile([F, SRC_CHUNK, BL], BF, tag="src", name=f"src{t}")
                        nc.gpsimd.dma_start(sc[:], srcT_d[:, t:t + SRC_CHUNK, :])
                    j = t % SRC_CHUNK
                    p, q = t % 2, (t + 1) % 2
                    cell([ew0_t], [sc[:, j, :]], eu0_t, h0b[p], h0b[q],
                         CT_ENC0, C4_ENC0, gx_first=True)
                    cell(ew1_t, [h0b[q][:, k, :] for k in range(KC)], eu1_t,
                         h1b[p], h1b[q], CT_ENC1, C4_ENC1, gx_first=False)
                sc_last = sc

                # ---------------- decoder ----------------
                for d in range(horizons):
                    p, q = (lags + d) % 2, (lags + d + 1) % 2
                    if d == 0:
                        cell([dw0_t], [sc_last[:, (lags - 1) % SRC_CHUNK, :]],
                             du0_t, h0b[p], h0b[q], CT_DEC0F, C4_DEC0, gx_first=True)
                    else:
                        cell(wcomb_t, [h1b[p][:, k, :] for k in range(KC)],
                             du0_t, h0b[p], h0b[q], CT_DEC0, C4_DEC0, gx_first=False)
                    cell(dw1_t, [h0b[q][:, k, :] for k in range(KC)], du1_t,
                         h1b[p], h1b[q], CT_DEC1, C4_DEC1, gx_first=False)
                    # out1[d] = W1 . h1_new   (b1 added on host)
                    po = pp.tile([128, 512], F32, tag="pA0", name=f"po{d}")
                    for k in range(KC):
                        nc.tensor.matmul(po[0:1, 0:BL], w1t_t[:, k:k + 1],
                                         h1b[q][:, k, :], start=(k == 0), stop=(k == KC - 1))
                    osb = opool.tile([1, BL], F32, tag="o1", name=f"o{d}")
                    nc.scalar.copy(osb[:], po[0:1, 0:BL])
                    nc.sync.dma_start(out_d[d:d + 1, :], osb[:])

    nc.compile()
    return nc


def _host_prep(inputs):
    import ml_dtypes
    f32 = np.float32
    bf16 = ml_dtypes.bfloat16
    g = {k: np.asarray(v, dtype=f32) for k, v in inputs.items()
         if k not in ("train",)}
    src = g["src"]
    eW0, eU0, eb0, ec0 = g["eW0"], g["eU0"], g["eb0"], g["ec0"]
    eW1, eU1, eb1, ec1 = g["eW1"], g["eU1"], g["eb1"], g["ec1"]
    dW0, dU0, db0, dc0 = g["dW0"], g["dU0"], g["db0"], g["dc0"]
    dW1, dU1, db1, dc1 = g["dW1"], g["dU1"], g["db1"], g["dc1"]
    W1, b1, W4, b4 = g["W1"], g["b1"], g["W4"], g["b4"]

    Wcomb = (dW0 @ W4).astype(f32)                       # [1536, 512]
    dcomb = (db0 + dW0 @ b4).astype(f32)                 # [1536]

    biases = np.zeros((128, 76), f32)
    rz_sets = [(eb0 + ec0), (eb1 + ec1), (db0 + dc0), (dcomb + dc0), (db1 + dc1)]
    for ct, s in enumerate(rz_sets):
        for gate, goff in ((0, 0), (1, H)):
            for m in range(KC):
                biases[:, ct * 8 + gate * 4 + m] = s[goff + m * 128:goff + (m + 1) * 128]
    bn_sets = [eb0, eb1, db0, dcomb, db1]
    for ct, s in enumerate(bn_sets):
        sn = s[2 * H:]
        for m in range(KC):
            biases[:, 40 + ct * 4 + m] = sn[m * 128:(m + 1) * 128]
    cn_sets = [ec0, ec1, dc0, dc1]
    for c4, s in enumerate(cn_sets):
        sn = s[2 * H:]
        for m in range(KC):
            biases[:, 60 + c4 * 4 + m] = sn[m * 128:(m + 1) * 128]

    shared = {
        "eu0": eU0.T.astype(bf16), "ew1": eW1.T.astype(bf16),
        "eu1": eU1.T.astype(bf16),
        "du0": dU0.T.astype(bf16), "dw1": dW1.T.astype(bf16),
        "du1": dU1.T.astype(bf16),
        "wcomb": Wcomb.T.astype(bf16),
        "ew0": eW0.T.astype(bf16), "dw0": dW0.T.astype(bf16),
        "biases": biases,
        "w1t": W1[0].reshape(KC, 128).T.astype(bf16),
    }
    shared = {k: np.ascontiguousarray(v) for k, v in shared.items()}

    in_maps = []
    for c in range(NCORES):
        s = src[c * BL:(c + 1) * BL]                     # [256, 64, 64]
        sT = np.ascontiguousarray(s.transpose(2, 1, 0).astype(bf16))
        m = dict(shared)
        m["srcT"] = sT
        in_maps.append(m)
    return in_maps, float(b1[0])


class _Runner:
    """Build-once sharded PJRT runner (axon: 8 NeuronCores)."""

    def __init__(self, nc):
        import jax
        from jax.sharding import Mesh, PartitionSpec
        from jax.experimental.shard_map import shard_map
        from concourse import mybir
        from concourse.bass2jax import (_bass_exec_p, partition_id_tensor,
                                        install_neuronx_cc_hook)
        install_neuronx_cc_hook()
        self.jax = jax
        partition_name = nc.partition_id_tensor.name if nc.partition_id_tensor else None
        in_names, out_names, out_avals, zero_outs = [], [], [], []
        for alloc in nc.m.functions[0].allocations:
            if not isinstance(alloc, mybir.MemoryLocationSet):
                continue
            name = alloc.memorylocations[0].name
            if alloc.kind == "ExternalInput":
                if name != partition_name:
                    in_names.append(name)
            elif alloc.kind == "ExternalOutput":
                out_names.append(name)
                shape = tuple(alloc.tensor_shape)
                dtype = mybir.dt.np(alloc.dtype)
                out_avals.append(jax.core.ShapedArray(shape, dtype))
                zero_outs.append(np.zeros(shape, dtype))
        n_params = len(in_names)
        all_in = list(in_names) + list(out_names)
        if partition_name is not None:
            all_in.append(partition_name)
        self.in_names, self.out_names = in_names, out_names
        self.out_avals, self.zero_outs = out_avals, zero_outs

        def _body(*args):
            operands = list(args)
            if partition_name is not None:
                operands.append(partition_id_tensor())
            return tuple(_bass_exec_p.bind(
                *operands, out_avals=tuple(out_avals), in_names=tuple(all_in),
                out_names=tuple(out_names), lowering_input_output_aliases=(),
                sim_require_finite=True, sim_require_nnan=True, nc=nc))

        devices = jax.devices()[:NCORES]
        self.mesh = Mesh(np.asarray(devices), ("core",))
        in_specs = (PartitionSpec("core"),) * (n_params + len(out_names))
        out_specs = (PartitionSpec("core"),) * len(out_names)
        donate = tuple(range(n_params, n_params + len(out_names)))
        self.fn = jax.jit(
            shard_map(_body, mesh=self.mesh, in_specs=in_specs,
                      out_specs=out_specs, check_rep=False),
            donate_argnums=donate, keep_unused=True)
        self.sh = jax.sharding.NamedSharding(self.mesh, PartitionSpec("core"))

    def place(self, in_maps):
        n = NCORES
        self.placed = [
            self.jax.device_put(np.ascontiguousarray(
                np.concatenate([in_maps[c][nm] for c in range(n)], 0)), self.sh)
            for nm in self.in_names]

    def run(self):
        zeros = [self.jax.device_put(
            np.zeros((NCORES * z.shape[0], *z.shape[1:]), z.dtype), self.sh)
            for z in self.zero_outs]
        outs = self.fn(*self.placed, *zeros)
        self.jax.block_until_ready(outs)
        return outs

    def results(self, outs):
        return [
            {nm: np.asarray(outs[i]).reshape(NCORES, *self.out_avals[i].shape)[c]
             for i, nm in enumerate(self.out_names)}
            for c in range(NCORES)]


def get_runner(repeat=1):
    global _RUNNER
    key = ("r2", repeat)
    if _RUNNER is None or _RUNNER[0] != key:
        nc = _build_nc(repeat=repeat)
        _RUNNER = (key, _Runner(nc))
    return _RUNNER[1]


def kernel(**inputs) -> np.ndarray:
    in_maps, b1 = _host_prep(inputs)
    r = get_runner()
    r.place(in_maps)
    res = r.results(r.run())
    out = np.empty((B, HORIZONS), np.float32)
    for c in range(NCORES):
        out[c * BL:(c + 1) * BL] = res[c]["out"].T + b1
    return out


# revision 8
# speedup vs baseline: 43.8977x; 43.1251x over previous
"""GRU Seq2Seq Trainium2 kernel (nn_GRU_Seq2Seq_83219286327778).

Strategy: data-parallel over batch (2048 -> 8 x 256), gate-major transposed
layout on-device ([hidden/gate dim on partitions, batch on free dim]) so the
recurrence needs no transposes. Matmuls in bf16 (fp32 PSUM accumulate) to make
the per-matmul LDWEIGHTS cheap enough to hide under the moving stream; all
weights SBUF-resident from the start; biases folded into activation bias APs
and fused DVE ops (no rank-1 bias matmuls); gh emitted before gx in cells
whose input comes from freshly-computed state so the PE never stalls.
fc4 feedback folded into the next step's gx via Wcomb = dW0 @ W4.
"""
import sys
sys.path.insert(0, "/opt/trn_rl_repo")
import numpy as np

B, LAGS, HORIZONS, F, H = 2048, 64, 24, 64, 512
NCORES = 8
BL = B // NCORES           # 256 batch per core
G3 = 3 * H                 # 1536
KC = H // 128              # 4 k-chunks
SRC_CHUNK = 8              # timesteps per src DMA

# bias column layout in the [128, 76] biases tensor
# rz: ct*8 + g*4 + m   (ct in 0..4, g: 0=r 1=z, m tile)    cols 0..39
# bn: 40 + ct*4 + m    (x-side n bias per celltype)        cols 40..59
# cn: 60 + c4*4 + m    (h-side n bias per U-set)           cols 60..75
CT_ENC0, CT_ENC1, CT_DEC0F, CT_DEC0, CT_DEC1 = range(5)
C4_ENC0, C4_ENC1, C4_DEC0, C4_DEC1 = range(4)

_RUNNER = None


def _build_nc(repeat=1, lags=LAGS, horizons=HORIZONS):
    import concourse.tile as tile
    from concourse import mybir, bacc

    F32 = mybir.dt.float32
    BF = mybir.dt.bfloat16
    AF = mybir.ActivationFunctionType
    OP = mybir.AluOpType

    nc = bacc.Bacc("TRN2", target_bir_lowering=False)

    srcT_d = nc.dram_tensor("srcT", [F, LAGS, BL], BF, kind="ExternalInput")
    wnames = ["eu0", "ew1", "eu1", "du0", "dw1", "du1", "wcomb"]
    w_d = {n: nc.dram_tensor(n, [H, G3], BF, kind="ExternalInput") for n in wnames}
    ew0_d = nc.dram_tensor("ew0", [F, G3], BF, kind="ExternalInput")
    dw0_d = nc.dram_tensor("dw0", [F, G3], BF, kind="ExternalInput")
    bias_d = nc.dram_tensor("biases", [128, 76], F32, kind="ExternalInput")
    w1t_d = nc.dram_tensor("w1t", [128, KC], BF, kind="ExternalInput")
    out_d = nc.dram_tensor("out", [HORIZONS, BL], F32, kind="ExternalOutput")

    with tile.TileContext(nc) as tc:
        with tc.tile_pool(name="wp", bufs=1) as wp, \
             tc.tile_pool(name="sp", bufs=2) as sp, \
             tc.tile_pool(name="hp", bufs=1) as hp, \
             tc.tile_pool(name="gp", bufs=2) as gp, \
             tc.tile_pool(name="op_", bufs=2) as opool, \
             tc.tile_pool(name="pp", bufs=1, space="PSUM") as pp:

            # ---- persistent small tensors ----
            bias_t = wp.tile([128, 76], F32, tag="bias", name="bias")
            nc.sync.dma_start(bias_t[:], bias_d[:])
            w1t_t = wp.tile([128, KC], BF, tag="w1t", name="w1t")
            nc.gpsimd.dma_start(w1t_t[:], w1t_d[:])
            ew0_t = wp.tile([F, G3], BF, tag="w0a", name="w0a")
            nc.gpsimd.dma_start(ew0_t[:], ew0_d[:])
            dw0_t = wp.tile([F, G3], BF, tag="dw0a", name="dw0a")
            nc.gpsimd.dma_start(dw0_t[:], dw0_d[:])

            def load_u(dram, tagbase):
                ts_ = []
                for k in range(KC):
                    t = wp.tile([128, G3], BF, tag=f"{tagbase}{k}", name=f"{tagbase}{k}")
                    nc.gpsimd.dma_start(t[:], dram[k * 128:(k + 1) * 128, :])
                    ts_.append(t)
                return ts_

            # all weights resident for the whole kernel
            eu0_t = load_u(w_d["eu0"], "uA")
            ew1_t = load_u(w_d["ew1"], "uB")
            eu1_t = load_u(w_d["eu1"], "uC")
            du0_t = load_u(w_d["du0"], "uD")
            dw1_t = load_u(w_d["dw1"], "uE")
            du1_t = load_u(w_d["du1"], "uF")
            wcomb_t = load_u(w_d["wcomb"], "uG")

            # hidden state ping-pong (bf16: matmul rhs + 2x DVE)
            h0b = [hp.tile([128, KC, BL], BF, tag=f"h0{i}", name=f"h0{i}") for i in range(2)]
            h1b = [hp.tile([128, KC, BL], BF, tag=f"h1{i}", name=f"h1{i}") for i in range(2)]

            def rzb(ct, g, m):
                c = ct * 8 + g * 4 + m
                return bias_t[:, c:c + 1]

            def bnb(ct, m):
                c = 40 + ct * 4 + m
                return bias_t[:, c:c + 1]

            def cnb(c4, m):
                c = 60 + c4 * 4 + m
                return bias_t[:, c:c + 1]

            def cell(gx_lhs, gx_rhs, gh_lhs, h_prev, h_out, ct, c4, gx_first):
                """One GRU cell step, gate-major.

                gx_lhs/gx_rhs: matching lists of lhsT tiles ([*, G3]) and rhs APs
                gh_lhs: KC lhsT tiles for the recurrent projection
                h_prev/h_out: [128, KC, BL] bf16 state tiles
                ct/c4: bias column groups; gx_first: emit gx phase before gh

                PSUM bank packing (one pending accumulation group per 2KB bank):
                pa[m] = r | hn, pb[m] = z | xn. The r/z groups stay open across
                the two phases; hn/xn are single-phase groups, ordered so each
                bank's groups are strictly sequential.
                """
                pa = [pp.tile([128, 512], F32, tag=f"pA{m}", name=f"pA{m}") for m in range(4)]
                pb = [pp.tile([128, 512], F32, tag=f"pB{m}", name=f"pB{m}") for m in range(4)]
                rz = gp.tile([128, 4, 512], BF, tag="rz", name="rz")
                tt = gp.tile([128, 4, BL], BF, tag="g1", name="tt")
                vv = gp.tile([128, 4, BL], BF, tag="g2", name="vv")
                nn = gp.tile([128, 4, BL], BF, tag="gn", name="nn")
                hp_chunks = [h_prev[:, k, :] for k in range(KC)]

                def emit(lhs_list, rhs_list, m, goff, out_ap, opening, closing):
                    n = len(lhs_list)
                    for i, (lhs, rhs) in enumerate(zip(lhs_list, rhs_list, strict=True)):
                        nc.tensor.matmul(
                            out_ap, lhs[:, goff + m * 128:goff + (m + 1) * 128], rhs,
                            start=(opening and i == 0), stop=(closing and i == n - 1))

                def epilogue(m):
                    # r/z/xn/hn for tile m all closed: sigmoids (+bias) + n folds
                    nc.scalar.activation(rz[:, m, 0:BL], pa[m][:, 0:BL],
                                         AF.Sigmoid, bias=rzb(ct, 0, m))
                    nc.scalar.activation(rz[:, m, BL:2 * BL], pb[m][:, 0:BL],
                                         AF.Sigmoid, bias=rzb(ct, 1, m))
                    # tt = (hn + cn) * r ; vv = (xn + bn) + tt
                    nc.vector.scalar_tensor_tensor(
                        tt[:, m, :], pa[m][:, BL:2 * BL], cnb(c4, m),
                        rz[:, m, 0:BL], OP.add, OP.mult)
                    nc.vector.scalar_tensor_tensor(
                        vv[:, m, :], pb[m][:, BL:2 * BL], bnb(ct, m),
                        tt[:, m, :], OP.add, OP.add)

                if gx_first:
                    for m in range(4):
                        emit(gx_lhs, gx_rhs, m, 1024, pb[m][:, BL:2 * BL], True, True)   # xn
                        emit(gx_lhs, gx_rhs, m, 0, pa[m][:, 0:BL], True, False)          # r open
                        emit(gx_lhs, gx_rhs, m, 512, pb[m][:, 0:BL], True, False)        # z open
                    for m in range(4):
                        emit(gh_lhs, hp_chunks, m, 0, pa[m][:, 0:BL], False, True)       # r close
                        emit(gh_lhs, hp_chunks, m, 512, pb[m][:, 0:BL], False, True)     # z close
                        emit(gh_lhs, hp_chunks, m, 1024, pa[m][:, BL:2 * BL], True, True)  # hn
                        epilogue(m)
                else:
                    for m in range(4):
                        emit(gh_lhs, hp_chunks, m, 1024, pa[m][:, BL:2 * BL], True, True)  # hn
                        emit(gh_lhs, hp_chunks, m, 0, pa[m][:, 0:BL], True, False)       # r open
                        emit(gh_lhs, hp_chunks, m, 512, pb[m][:, 0:BL], True, False)     # z open
                    for m in range(4):
                        emit(gx_lhs, gx_rhs, m, 0, pa[m][:, 0:BL], False, True)          # r close
                        emit(gx_lhs, gx_rhs, m, 512, pb[m][:, 0:BL], False, True)        # z close
                        emit(gx_lhs, gx_rhs, m, 1024, pb[m][:, BL:2 * BL], True, True)   # xn
                        epilogue(m)

                nc.scalar.activation(nn[:, :, :], vv[:, :, :], AF.Tanh)
                # h' = n + z*(h - n)
                dd = gp.tile([128, 4, BL], BF, tag="g1", name="dd")
                ee = gp.tile([128, 4, BL], BF, tag="g2", name="ee")
                nc.vector.tensor_tensor(dd[:], h_prev[:, 0:KC, :], nn[:], OP.subtract)
                nc.vector.tensor_tensor(ee[:], rz[:, :, BL:2 * BL], dd[:], OP.mult)
                nc.vector.tensor_tensor(h_out[:, 0:KC, :], ee[:], nn[:], OP.add)

            for _rep in range(repeat):
                for i in range(2):
                    nc.vector.memzero(h0b[i][:])
                    nc.vector.memzero(h1b[i][:])

                # ---------------- encoder ----------------
                sc = None
                for t in range(lags):
                    if t % SRC_CHUNK == 0:
                        sc = sp.tile([F, SRC_CHUNK, BL], BF, tag="src", name=f"src{t}")
                        nc.gpsimd.dma_start(sc[:], srcT_d[:, t:t + SRC_CHUNK, :])
                    j = t % SRC_CHUNK
                    p, q = t % 2, (t + 1) % 2
                    cell([ew0_t], [sc[:, j, :]], eu0_t, h0b[p], h0b[q],
                         CT_ENC0, C4_ENC0, gx_first=True)
                    cell(ew1_t, [h0b[q][:, k, :] for k in range(KC)], eu1_t,
                         h1b[p], h1b[q], CT_ENC1, C4_ENC1, gx_first=False)
                sc_last = sc

                # ---------------- decoder ----------------
                for d in range(horizons):
                    p, q = (lags + d) % 2, (lags + d + 1) % 2
                    if d == 0:
                        cell([dw0_t], [sc_last[:, (lags - 1) % SRC_CHUNK, :]],
                             du0_t, h0b[p], h0b[q], CT_DEC0F, C4_DEC0, gx_first=True)
                    else:
                        cell(wcomb_t, [h1b[p][:, k, :] for k in range(KC)],
                             du0_t, h0b[p], h0b[q], CT_DEC0, C4_DEC0, gx_first=False)
                    cell(dw1_t, [h0b[q][:, k, :] for k in range(KC)], du1_t,
                         h1b[p], h1b[q], CT_DEC1, C4_DEC1, gx_first=False)
                    # out1[d] = W1 . h1_new   (b1 added on host)
                    po = pp.tile([128, 512], F32, tag="pA0", name=f"po{d}")
                    for k in range(KC):
                        nc.tensor.matmul(po[0:1, 0:BL], w1t_t[:, k:k + 1],
                                         h1b[q][:, k, :], start=(k == 0), stop=(k == KC - 1))
                    osb = opool.tile([1, BL], F32, tag="o1", name=f"o{d}")
                    nc.scalar.copy(osb[:], po[0:1, 0:BL])
                    nc.sync.dma_start(out_d[d:d + 1, :], osb[:])

    nc.compile()
    return nc


def _host_prep(inputs):
    import ml_dtypes
    f32 = np.float32
    bf16 = ml_dtypes.bfloat16
    g = {k: np.asarray(v, dtype=f32) for k, v in inputs.items()
         if k not in ("train",)}
    src = g["src"]
    eW0, eU0, eb0, ec0 = g["eW0"], g["eU0"], g["eb0"], g["ec0"]
    eW1, eU1, eb1, ec1 = g["eW1"], g["eU1"], g["eb1"], g["ec1"]
    dW0, dU0, db0, dc0 = g["dW0"], g["dU0"], g["db0"], g["dc0"]
    dW1, dU1, db1, dc1 = g["dW1"], g["dU1"], g["db1"], g["dc1"]
    W1, b1, W4, b4 = g["W1"], g["b1"], g["W4"], g["b4"]

    Wcomb = (dW0 @ W4).astype(f32)                       # [1536, 512]
    dcomb = (db0 + dW0 @ b4).astype(f32)                 # [1536]

    biases = np.zeros((128, 76), f32)
    rz_sets = [(eb0 + ec0), (eb1 + ec1), (db0 + dc0), (dcomb + dc0), (db1 + dc1)]
    for ct, s in enumerate(rz_sets):
        for gate, goff in ((0, 0), (1, H)):
            for m in range(KC):
                biases[:, ct * 8 + gate * 4 + m] = s[goff + m * 128:goff + (m + 1) * 128]
    bn_sets = [eb0, eb1, db0, dcomb, db1]
    for ct, s in enumerate(bn_sets):
        sn = s[2 * H:]
        for m in range(KC):
            biases[:, 40 + ct * 4 + m] = sn[m * 128:(m + 1) * 128]
    cn_sets = [ec0, ec1, dc0, dc1]
    for c4, s in enumerate(cn_sets):
        sn = s[2 * H:]
        for m in range(KC):
            biases[:, 60 + c4 * 4 + m] = sn[m * 128:(m + 1) * 128]

    shared = {
        "eu0": eU0.T.astype(bf16), "ew1": eW1.T.astype(bf16),
        "eu1": eU1.T.astype(bf16),
        "du0": dU0.T.astype(bf16), "dw1": dW1.T.astype(bf16),
        "du1": dU1.T.astype(bf16),
        "wcomb": Wcomb.T.astype(bf16),
        "ew0": eW0.T.astype(bf16), "dw0": dW0.T.astype(bf16),
        "biases": biases,
        "w1t": W1[0].reshape(KC, 128).T.astype(bf16),
    }
    shared = {k: np.ascontiguousarray(v) for k, v in shared.items()}

    in_maps = []
    for c in range(NCORES):
        s = src[c * BL:(c + 1) * BL]                     # [256, 64, 64]
        sT = np.ascontiguousarray(s.transpose(2, 1, 0).astype(bf16))
        m = dict(shared)
        m["srcT"] = sT
        in_maps.append(m)
    return in_maps, float(b1[0])


class _Runner:
    """Build-once sharded PJRT runner (axon: 8 NeuronCores)."""

    def __init__(self, nc):
        import jax
        from jax.sharding import Mesh, PartitionSpec
        from jax.experimental.shard_map import shard_map
        from concourse import mybir
        from concourse.bass2jax import (_bass_exec_p, partition_id_tensor,
                                        install_neuronx_cc_hook)
        install_neuronx_cc_hook()
        self.jax = jax
        partition_name = nc.partition_id_tensor.name if nc.partition_id_tensor else None
        in_names, out_names, out_avals, zero_outs = [], [], [], []
        for alloc in nc.m.functions[0].allocations:
            if not isinstance(alloc, mybir.MemoryLocationSet):
                continue
            name = alloc.memorylocations[0].name
            if alloc.kind == "ExternalInput":
                if name != partition_name:
                    in_names.append(name)
            elif alloc.kind == "ExternalOutput":
                out_names.append(name)
                shape = tuple(alloc.tensor_shape)
                dtype = mybir.dt.np(alloc.dtype)
                out_avals.append(jax.core.ShapedArray(shape, dtype))
                zero_outs.append(np.zeros(shape, dtype))
        n_params = len(in_names)
        all_in = list(in_names) + list(out_names)
        if partition_name is not None:
            all_in.append(partition_name)
        self.in_names, self.out_names = in_names, out_names
        self.out_avals, self.zero_outs = out_avals, zero_outs

        def _body(*args):
            operands = list(args)
            if partition_name is not None:
                operands.append(partition_id_tensor())
            return tuple(_bass_exec_p.bind(
                *operands, out_avals=tuple(out_avals), in_names=tuple(all_in),
                out_names=tuple(out_names), lowering_input_output_aliases=(),
                sim_require_finite=True, sim_require_nnan=True, nc=nc))

        devices = jax.devices()[:NCORES]
        self.mesh = Mesh(np.asarray(devices), ("core",))
        in_specs = (PartitionSpec("core"),) * (n_params + len(out_names))
        out_specs = (PartitionSpec("core"),) * len(out_names)
        donate = tuple(range(n_params, n_params + len(out_names)))
        self.fn = jax.jit(
            shard_map(_body, mesh=self.mesh, in_specs=in_specs,
                      out_specs=out_specs, check_rep=False),
            donate_argnums=donate, keep_unused=True)
        self.sh = jax.sharding.NamedSharding(self.mesh, PartitionSpec("core"))

    def place(self, in_maps):
        n = NCORES
        self.placed = [
            self.jax.device_put(np.ascontiguousarray(
                np.concatenate([in_maps[c][nm] for c in range(n)], 0)), self.sh)
            for nm in self.in_names]

    def run(self):
        zeros = [self.jax.device_put(
            np.zeros((NCORES * z.shape[0], *z.shape[1:]), z.dtype), self.sh)
            for z in self.zero_outs]
        outs = self.fn(*self.placed, *zeros)
        self.jax.block_until_ready(outs)
        return outs

    def results(self, outs):
        return [
            {nm: np.asarray(outs[i]).reshape(NCORES, *self.out_avals[i].shape)[c]
             for i, nm in enumerate(self.out_names)}
            for c in range(NCORES)]


def get_runner(repeat=1):
    global _RUNNER
    key = ("r2", repeat)
    if _RUNNER is None or _RUNNER[0] != key:
        nc = _build_nc(repeat=repeat)
        _RUNNER = (key, _Runner(nc))
    return _RUNNER[1]


def kernel(**inputs) -> np.ndarray:
    in_maps, b1 = _host_prep(inputs)
    r = get_runner()
    r.place(in_maps)
    res = r.results(r.run())
    out = np.empty((B, HORIZONS), np.float32)
    for c in range(NCORES):
        out[c * BL:(c + 1) * BL] = res[c]["out"].T + b1
    return out


# revision 12
# speedup vs baseline: 44.9214x; 1.0233x over previous
"""GRU Seq2Seq Trainium2 kernel (nn_GRU_Seq2Seq_83219286327778).

Strategy: data-parallel over batch (2048 -> 8 x 256), gate-major transposed
layout on-device ([hidden/gate dim on partitions, batch on free dim]) so the
recurrence needs no transposes. Matmuls in bf16 (fp32 PSUM accumulate) to make
the per-matmul LDWEIGHTS cheap enough to hide under the moving stream; all
weights SBUF-resident from the start; biases folded into activation bias APs
and fused DVE ops (no rank-1 bias matmuls); gh emitted before gx in cells
whose input comes from freshly-computed state so the PE never stalls.
fc4 feedback folded into the next step's gx via Wcomb = dW0 @ W4.
"""
import sys
sys.path.insert(0, "/opt/trn_rl_repo")
import numpy as np

B, LAGS, HORIZONS, F, H = 2048, 64, 24, 64, 512
NCORES = 8
BL = B // NCORES           # 256 batch per core
G3 = 3 * H                 # 1536
KC = H // 128              # 4 k-chunks
SRC_CHUNK = 8              # timesteps per src DMA

# bias column layout in the [128, 76] biases tensor
# rz: ct*8 + g*4 + m   (ct in 0..4, g: 0=r 1=z, m tile)    cols 0..39
# bn: 40 + ct*4 + m    (x-side n bias per celltype)        cols 40..59
# cn: 60 + c4*4 + m    (h-side n bias per U-set)           cols 60..75
CT_ENC0, CT_ENC1, CT_DEC0F, CT_DEC0, CT_DEC1 = range(5)
C4_ENC0, C4_ENC1, C4_DEC0, C4_DEC1 = range(4)

_RUNNER = None


def _build_nc(repeat=1, lags=LAGS, horizons=HORIZONS):
    import concourse.tile as tile
    from concourse import mybir, bacc

    F32 = mybir.dt.float32
    BF = mybir.dt.bfloat16
    AF = mybir.ActivationFunctionType
    OP = mybir.AluOpType

    nc = bacc.Bacc("TRN2", target_bir_lowering=False)

    srcT_d = nc.dram_tensor("srcT", [F, LAGS, BL], BF, kind="ExternalInput")
    wnames = ["eu0", "ew1", "eu1", "du0", "dw1", "du1", "wcomb"]
    w_d = {n: nc.dram_tensor(n, [H, G3], BF, kind="ExternalInput") for n in wnames}
    ew0_d = nc.dram_tensor("ew0", [F, G3], BF, kind="ExternalInput")
    dw0_d = nc.dram_tensor("dw0", [F, G3], BF, kind="ExternalInput")
    bias_d = nc.dram_tensor("biases", [128, 76], F32, kind="ExternalInput")
    w1t_d = nc.dram_tensor("w1t", [128, KC], BF, kind="ExternalInput")
    out_d = nc.dram_tensor("out", [HORIZONS, BL], F32, kind="ExternalOutput")

    with tile.TileContext(nc) as tc:
        with tc.tile_pool(name="wp", bufs=1) as wp, \
             tc.tile_pool(name="sp", bufs=2) as sp, \
             tc.tile_pool(name="hp", bufs=1) as hp, \
             tc.tile_pool(name="gp", bufs=2) as gp, \
             tc.tile_pool(name="op_", bufs=2) as opool, \
             tc.tile_pool(name="pp", bufs=1, space="PSUM") as pp:

            # ---- persistent small tensors ----
            # DMA issue order = queue order: first-needed tensors first so the
            # first cell isn't stuck behind the 10MB weight stream. src chunks
            # ride the (idle) sync queue, concurrent with the gpsimd stream.
            bias_t = wp.tile([128, 76], F32, tag="bias", name="bias")
            nc.sync.dma_start(bias_t[:], bias_d[:])
            ew0_t = wp.tile([F, G3], BF, tag="w0a", name="w0a")
            nc.gpsimd.dma_start(ew0_t[:], ew0_d[:])

            def load_u(dram, tagbase):
                ts_ = []
                for k in range(KC):
                    t = wp.tile([128, G3], BF, tag=f"{tagbase}{k}", name=f"{tagbase}{k}")
                    nc.gpsimd.dma_start(t[:], dram[k * 128:(k + 1) * 128, :])
                    ts_.append(t)
                return ts_

            # all weights resident for the whole kernel, in first-use order
            eu0_t = load_u(w_d["eu0"], "uA")
            ew1_t = load_u(w_d["ew1"], "uB")
            eu1_t = load_u(w_d["eu1"], "uC")
            du0_t = load_u(w_d["du0"], "uD")
            dw1_t = load_u(w_d["dw1"], "uE")
            du1_t = load_u(w_d["du1"], "uF")
            wcomb_t = load_u(w_d["wcomb"], "uG")
            dw0_t = wp.tile([F, G3], BF, tag="dw0a", name="dw0a")
            nc.gpsimd.dma_start(dw0_t[:], dw0_d[:])
            w1t_t = wp.tile([128, KC], BF, tag="w1t", name="w1t")
            nc.gpsimd.dma_start(w1t_t[:], w1t_d[:])

            # hidden state ping-pong (bf16: matmul rhs + 2x DVE)
            h0b = [hp.tile([128, KC, BL], BF, tag=f"h0{i}", name=f"h0{i}") for i in range(2)]
            h1b = [hp.tile([128, KC, BL], BF, tag=f"h1{i}", name=f"h1{i}") for i in range(2)]

            def rzb(ct, g, m):
                c = ct * 8 + g * 4 + m
                return bias_t[:, c:c + 1]

            def bnb(ct, m):
                c = 40 + ct * 4 + m
                return bias_t[:, c:c + 1]

            def cnb(c4, m):
                c = 60 + c4 * 4 + m
                return bias_t[:, c:c + 1]

            def cell(gx_lhs, gx_rhs, gh_lhs, h_prev, h_out, ct, c4, gx_first,
                     first=False):
                """One GRU cell step, gate-major.

                gx_lhs/gx_rhs: matching lists of lhsT tiles ([*, G3]) and rhs APs
                gh_lhs: KC lhsT tiles for the recurrent projection
                h_prev/h_out: [128, KC, BL] bf16 state tiles
                ct/c4: bias column groups; gx_first: emit gx phase before gh
                first: h_prev is all zeros -- skip the gh/hn matmuls entirely

                PSUM bank packing (one pending accumulation group per 2KB bank):
                pa[m] = r | hn, pb[m] = z | xn. The r/z groups stay open across
                the two phases; hn/xn are single-phase groups, ordered so each
                bank's groups are strictly sequential.
                """
                pa = [pp.tile([128, 512], F32, tag=f"pA{m}", name=f"pA{m}") for m in range(4)]
                pb = [pp.tile([128, 512], F32, tag=f"pB{m}", name=f"pB{m}") for m in range(4)]
                rz = gp.tile([128, 4, 512], BF, tag="rz", name="rz")
                tt = gp.tile([128, 4, BL], BF, tag="g1", name="tt")
                vv = gp.tile([128, 4, BL], BF, tag="g2", name="vv")
                nn = gp.tile([128, 4, BL], BF, tag="gn", name="nn")
                hp_chunks = [h_prev[:, k, :] for k in range(KC)]

                def emit(lhs_list, rhs_list, m, goff, out_ap, opening, closing):
                    n = len(lhs_list)
                    for i, (lhs, rhs) in enumerate(zip(lhs_list, rhs_list, strict=True)):
                        nc.tensor.matmul(
                            out_ap, lhs[:, goff + m * 128:goff + (m + 1) * 128], rhs,
                            start=(opening and i == 0), stop=(closing and i == n - 1))

                def epilogue(m):
                    # r/z/xn/hn for tile m all closed: sigmoids (+bias) + n folds
                    nc.scalar.activation(rz[:, m, 0:BL], pa[m][:, 0:BL],
                                         AF.Sigmoid, bias=rzb(ct, 0, m))
                    nc.scalar.activation(rz[:, m, BL:2 * BL], pb[m][:, 0:BL],
                                         AF.Sigmoid, bias=rzb(ct, 1, m))
                    # tt = (hn + cn) * r ; vv = (xn + bn) + tt
                    nc.vector.scalar_tensor_tensor(
                        tt[:, m, :], pa[m][:, BL:2 * BL], cnb(c4, m),
                        rz[:, m, 0:BL], OP.add, OP.mult)
                    nc.vector.scalar_tensor_tensor(
                        vv[:, m, :], pb[m][:, BL:2 * BL], bnb(ct, m),
                        tt[:, m, :], OP.add, OP.add)

                if first:
                    for m in range(4):
                        emit(gx_lhs, gx_rhs, m, 1024, pb[m][:, BL:2 * BL], True, True)   # xn
                        emit(gx_lhs, gx_rhs, m, 0, pa[m][:, 0:BL], True, True)           # r
                        emit(gx_lhs, gx_rhs, m, 512, pb[m][:, 0:BL], True, True)         # z
                        # gh == 0: tt = r * cn ; vv = (xn + bn) + tt
                        nc.scalar.activation(rz[:, m, 0:BL], pa[m][:, 0:BL],
                                             AF.Sigmoid, bias=rzb(ct, 0, m))
                        nc.scalar.activation(rz[:, m, BL:2 * BL], pb[m][:, 0:BL],
                                             AF.Sigmoid, bias=rzb(ct, 1, m))
                        nc.vector.tensor_scalar(
                            tt[:, m, :], rz[:, m, 0:BL], cnb(c4, m), None, OP.mult)
                        nc.vector.scalar_tensor_tensor(
                            vv[:, m, :], pb[m][:, BL:2 * BL], bnb(ct, m),
                            tt[:, m, :], OP.add, OP.add)
                elif gx_first:
                    for m in range(4):
                        emit(gx_lhs, gx_rhs, m, 1024, pb[m][:, BL:2 * BL], True, True)   # xn
                        emit(gx_lhs, gx_rhs, m, 0, pa[m][:, 0:BL], True, False)          # r open
                        emit(gx_lhs, gx_rhs, m, 512, pb[m][:, 0:BL], True, False)        # z open
                    for m in range(4):
                        emit(gh_lhs, hp_chunks, m, 0, pa[m][:, 0:BL], False, True)       # r close
                        emit(gh_lhs, hp_chunks, m, 512, pb[m][:, 0:BL], False, True)     # z close
                        emit(gh_lhs, hp_chunks, m, 1024, pa[m][:, BL:2 * BL], True, True)  # hn
                        epilogue(m)
                else:
                    for m in range(4):
                        emit(gh_lhs, hp_chunks, m, 1024, pa[m][:, BL:2 * BL], True, True)  # hn
                        emit(gh_lhs, hp_chunks, m, 0, pa[m][:, 0:BL], True, False)       # r open
                        emit(gh_lhs, hp_chunks, m, 512, pb[m][:, 0:BL], True, False)     # z open
                    for m in range(4):
                        emit(gx_lhs, gx_rhs, m, 0, pa[m][:, 0:BL], False, True)          # r close
                        emit(gx_lhs, gx_rhs, m, 512, pb[m][:, 0:BL], False, True)        # z close
                        emit(gx_lhs, gx_rhs, m, 1024, pb[m][:, BL:2 * BL], True, True)   # xn
                        epilogue(m)

                nc.scalar.activation(nn[:, :, :], vv[:, :, :], AF.Tanh)
                # h' = n + z*(h - n)
                dd = gp.tile([128, 4, BL], BF, tag="g1", name="dd")
                ee = gp.tile([128, 4, BL], BF, tag="g2", name="ee")
                nc.vector.tensor_tensor(dd[:], h_prev[:, 0:KC, :], nn[:], OP.subtract)
                nc.vector.tensor_tensor(ee[:], rz[:, :, BL:2 * BL], dd[:], OP.mult)
                nc.vector.tensor_tensor(h_out[:, 0:KC, :], ee[:], nn[:], OP.add)

            for _rep in range(repeat):
                for i in range(2):
                    nc.vector.memzero(h0b[i][:])
                    nc.vector.memzero(h1b[i][:])

                # ---------------- encoder ----------------
                sc = None
                for t in range(lags):
                    if t % SRC_CHUNK == 0:
                        sc = sp.tile([F, SRC_CHUNK, BL], BF, tag="src", name=f"src{t}")
                        nc.sync.dma_start(sc[:], srcT_d[:, t:t + SRC_CHUNK, :])
                    j = t % SRC_CHUNK
                    p, q = t % 2, (t + 1) % 2
                    cell([ew0_t], [sc[:, j, :]], eu0_t, h0b[p], h0b[q],
                         CT_ENC0, C4_ENC0, gx_first=True, first=(t == 0))
                    cell(ew1_t, [h0b[q][:, k, :] for k in range(KC)], eu1_t,
                         h1b[p], h1b[q], CT_ENC1, C4_ENC1, gx_first=False,
                         first=(t == 0))
                sc_last = sc

                # ---------------- decoder ----------------
                for d in range(horizons):
                    p, q = (lags + d) % 2, (lags + d + 1) % 2
                    if d == 0:
                        cell([dw0_t], [sc_last[:, (lags - 1) % SRC_CHUNK, :]],
                             du0_t, h0b[p], h0b[q], CT_DEC0F, C4_DEC0, gx_first=True)
                    else:
                        cell(wcomb_t, [h1b[p][:, k, :] for k in range(KC)],
                             du0_t, h0b[p], h0b[q], CT_DEC0, C4_DEC0, gx_first=False)
                    cell(dw1_t, [h0b[q][:, k, :] for k in range(KC)], du1_t,
                         h1b[p], h1b[q], CT_DEC1, C4_DEC1, gx_first=False)
                    # out1[d] = W1 . h1_new   (b1 added on host)
                    po = pp.tile([128, 512], F32, tag="pA0", name=f"po{d}")
                    for k in range(KC):
                        nc.tensor.matmul(po[0:1, 0:BL], w1t_t[:, k:k + 1],
                                         h1b[q][:, k, :], start=(k == 0), stop=(k == KC - 1))
                    osb = opool.tile([1, BL], F32, tag="o1", name=f"o{d}")
                    nc.scalar.copy(osb[:], po[0:1, 0:BL])
                    nc.sync.dma_start(out_d[d:d + 1, :], osb[:])

    nc.compile()
    return nc


def _host_prep(inputs):
    import ml_dtypes
    f32 = np.float32
    bf16 = ml_dtypes.bfloat16
    g = {k: np.asarray(v, dtype=f32) for k, v in inputs.items()
         if k not in ("train",)}
    src = g["src"]
    eW0, eU0, eb0, ec0 = g["eW0"], g["eU0"], g["eb0"], g["ec0"]
    eW1, eU1, eb1, ec1 = g["eW1"], g["eU1"], g["eb1"], g["ec1"]
    dW0, dU0, db0, dc0 = g["dW0"], g["dU0"], g["db0"], g["dc0"]
    dW1, dU1, db1, dc1 = g["dW1"], g["dU1"], g["db1"], g["dc1"]
    W1, b1, W4, b4 = g["W1"], g["b1"], g["W4"], g["b4"]

    Wcomb = (dW0 @ W4).astype(f32)                       # [1536, 512]
    dcomb = (db0 + dW0 @ b4).astype(f32)                 # [1536]

    biases = np.zeros((128, 76), f32)
    rz_sets = [(eb0 + ec0), (eb1 + ec1), (db0 + dc0), (dcomb + dc0), (db1 + dc1)]
    for ct, s in enumerate(rz_sets):
        for gate, goff in ((0, 0), (1, H)):
            for m in range(KC):
                biases[:, ct * 8 + gate * 4 + m] = s[goff + m * 128:goff + (m + 1) * 128]
    bn_sets = [eb0, eb1, db0, dcomb, db1]
    for ct, s in enumerate(bn_sets):
        sn = s[2 * H:]
        for m in range(KC):
            biases[:, 40 + ct * 4 + m] = sn[m * 128:(m + 1) * 128]
    cn_sets = [ec0, ec1, dc0, dc1]
    for c4, s in enumerate(cn_sets):
        sn = s[2 * H:]
        for m in range(KC):
            biases[:, 60 + c4 * 4 + m] = sn[m * 128:(m + 1) * 128]

    shared = {
        "eu0": eU0.T.astype(bf16), "ew1": eW1.T.astype(bf16),
        "eu1": eU1.T.astype(bf16),
        "du0": dU0.T.astype(bf16), "dw1": dW1.T.astype(bf16),
        "du1": dU1.T.astype(bf16),
        "wcomb": Wcomb.T.astype(bf16),
        "ew0": eW0.T.astype(bf16), "dw0": dW0.T.astype(bf16),
        "biases": biases,
        "w1t": W1[0].reshape(KC, 128).T.astype(bf16),
    }
    shared = {k: np.ascontiguousarray(v) for k, v in shared.items()}

    in_maps = []
    for c in range(NCORES):
        s = src[c * BL:(c + 1) * BL]                     # [256, 64, 64]
        sT = np.ascontiguousarray(s.transpose(2, 1, 0).astype(bf16))
        m = dict(shared)
        m["srcT"] = sT
        in_maps.append(m)
    return in_maps, float(b1[0])


class _Runner:
    """Build-once sharded PJRT runner (axon: 8 NeuronCores)."""

    def __init__(self, nc):
        import jax
        from jax.sharding import Mesh, PartitionSpec
        from jax.experimental.shard_map import shard_map
        from concourse import mybir
        from concourse.bass2jax import (_bass_exec_p, partition_id_tensor,
                                        install_neuronx_cc_hook)
        install_neuronx_cc_hook()
        self.jax = jax
        partition_name = nc.partition_id_tensor.name if nc.partition_id_tensor else None
        in_names, out_names, out_avals, zero_outs = [], [], [], []
        for alloc in nc.m.functions[0].allocations:
            if not isinstance(alloc, mybir.MemoryLocationSet):
                continue
            name = alloc.memorylocations[0].name
            if alloc.kind == "ExternalInput":
                if name != partition_name:
                    in_names.append(name)
            elif alloc.kind == "ExternalOutput":
                out_names.append(name)
                shape = tuple(alloc.tensor_shape)
                dtype = mybir.dt.np(alloc.dtype)
                out_avals.append(jax.core.ShapedArray(shape, dtype))
                zero_outs.append(np.zeros(shape, dtype))
        n_params = len(in_names)
        all_in = list(in_names) + list(out_names)
        if partition_name is not None:
            all_in.append(partition_name)
        self.in_names, self.out_names = in_names, out_names
        self.out_avals, self.zero_outs = out_avals, zero_outs

        def _body(*args):
            operands = list(args)
            if partition_name is not None:
                operands.append(partition_id_tensor())
            return tuple(_bass_exec_p.bind(
                *operands, out_avals=tuple(out_avals), in_names=tuple(all_in),
                out_names=tuple(out_names), lowering_input_output_aliases=(),
                sim_require_finite=True, sim_require_nnan=True, nc=nc))

        devices = jax.devices()[:NCORES]
        self.mesh = Mesh(np.asarray(devices), ("core",))
        in_specs = (PartitionSpec("core"),) * (n_params + len(out_names))
        out_specs = (PartitionSpec("core"),) * len(out_names)
        donate = tuple(range(n_params, n_params + len(out_names)))
        self.fn = jax.jit(
            shard_map(_body, mesh=self.mesh, in_specs=in_specs,
                      out_specs=out_specs, check_rep=False),
            donate_argnums=donate, keep_unused=True)
        self.sh = jax.sharding.NamedSharding(self.mesh, PartitionSpec("core"))

    def place(self, in_maps):
        n = NCORES
        self.placed = [
            self.jax.device_put(np.ascontiguousarray(
                np.concatenate([in_maps[c][nm] for c in range(n)], 0)), self.sh)
            for nm in self.in_names]

    def run(self):
        zeros = [self.jax.device_put(
            np.zeros((NCORES * z.shape[0], *z.shape[1:]), z.dtype), self.sh)
            for z in self.zero_outs]
        outs = self.fn(*self.placed, *zeros)
        self.jax.block_until_ready(outs)
        return outs

    def results(self, outs):
        return [
            {nm: np.asarray(outs[i]).reshape(NCORES, *self.out_avals[i].shape)[c]
             for i, nm in enumerate(self.out_names)}
            for c in range(NCORES)]


def get_runner(repeat=1):
    global _RUNNER
    key = ("r2", repeat)
    if _RUNNER is None or _RUNNER[0] != key:
        nc = _build_nc(repeat=repeat)
        _RUNNER = (key, _Runner(nc))
    return _RUNNER[1]


def kernel(**inputs) -> np.ndarray:
    in_maps, b1 = _host_prep(inputs)
    r = get_runner()
    r.place(in_maps)
    res = r.results(r.run())
    out = np.empty((B, HORIZONS), np.float32)
    for c in range(NCORES):
        out[c * BL:(c + 1) * BL] = res[c]["out"].T + b1
    return out


# revision 13
# speedup vs baseline: 44.9401x; 1.0004x over previous
"""GRU Seq2Seq Trainium2 kernel (nn_GRU_Seq2Seq_83219286327778).

Strategy: data-parallel over batch (2048 -> 8 x 256), gate-major transposed
layout on-device ([hidden/gate dim on partitions, batch on free dim]) so the
recurrence needs no transposes. Matmuls in bf16 (fp32 PSUM accumulate) to make
the per-matmul LDWEIGHTS cheap enough to hide under the moving stream; all
weights SBUF-resident from the start; biases folded into activation bias APs
and fused DVE ops (no rank-1 bias matmuls); gh emitted before gx in cells
whose input comes from freshly-computed state so the PE never stalls.
fc4 feedback folded into the next step's gx via Wcomb = dW0 @ W4.
"""
import sys
sys.path.insert(0, "/opt/trn_rl_repo")
import numpy as np

B, LAGS, HORIZONS, F, H = 2048, 64, 24, 64, 512
NCORES = 8
BL = B // NCORES           # 256 batch per core
G3 = 3 * H                 # 1536
KC = H // 128              # 4 k-chunks
SRC_CHUNK = 8              # timesteps per src DMA

# bias column layout in the [128, 76] biases tensor
# rz: ct*8 + g*4 + m   (ct in 0..4, g: 0=r 1=z, m tile)    cols 0..39
# bn: 40 + ct*4 + m    (x-side n bias per celltype)        cols 40..59
# cn: 60 + c4*4 + m    (h-side n bias per U-set)           cols 60..75
CT_ENC0, CT_ENC1, CT_DEC0F, CT_DEC0, CT_DEC1 = range(5)
C4_ENC0, C4_ENC1, C4_DEC0, C4_DEC1 = range(4)

_RUNNER = None


def _build_nc(repeat=1, lags=LAGS, horizons=HORIZONS):
    import concourse.tile as tile
    from concourse import mybir, bacc

    F32 = mybir.dt.float32
    BF = mybir.dt.bfloat16
    AF = mybir.ActivationFunctionType
    OP = mybir.AluOpType

    nc = bacc.Bacc("TRN2", target_bir_lowering=False)

    srcT_d = nc.dram_tensor("srcT", [F, LAGS, BL], BF, kind="ExternalInput")
    wnames = ["eu0", "ew1", "eu1", "du0", "dw1", "du1", "wcomb"]
    w_d = {n: nc.dram_tensor(n, [H, G3], BF, kind="ExternalInput") for n in wnames}
    ew0_d = nc.dram_tensor("ew0", [F, G3], BF, kind="ExternalInput")
    dw0_d = nc.dram_tensor("dw0", [F, G3], BF, kind="ExternalInput")
    bias_d = nc.dram_tensor("biases", [128, 76], F32, kind="ExternalInput")
    w1t_d = nc.dram_tensor("w1t", [128, KC], BF, kind="ExternalInput")
    out_d = nc.dram_tensor("out", [HORIZONS, BL], F32, kind="ExternalOutput")

    with tile.TileContext(nc) as tc:
        with tc.tile_pool(name="wp", bufs=1) as wp, \
             tc.tile_pool(name="sp", bufs=2) as sp, \
             tc.tile_pool(name="hp", bufs=1) as hp, \
             tc.tile_pool(name="gp", bufs=2) as gp, \
             tc.tile_pool(name="op_", bufs=2) as opool, \
             tc.tile_pool(name="pp", bufs=1, space="PSUM") as pp:

            # ---- persistent small tensors ----
            # DMA issue order = queue order: first-needed tensors first so the
            # first cell isn't stuck behind the 10MB weight stream. src chunks
            # ride the (idle) sync queue, concurrent with the gpsimd stream.
            bias_t = wp.tile([128, 76], F32, tag="bias", name="bias")
            nc.sync.dma_start(bias_t[:], bias_d[:])
            ew0_t = wp.tile([F, G3], BF, tag="w0a", name="w0a")
            nc.gpsimd.dma_start(ew0_t[:], ew0_d[:])

            def load_u(dram, tagbase):
                ts_ = []
                for k in range(KC):
                    t = wp.tile([128, G3], BF, tag=f"{tagbase}{k}", name=f"{tagbase}{k}")
                    nc.gpsimd.dma_start(t[:], dram[k * 128:(k + 1) * 128, :])
                    ts_.append(t)
                return ts_

            # all weights resident for the whole kernel, in first-use order
            eu0_t = load_u(w_d["eu0"], "uA")
            ew1_t = load_u(w_d["ew1"], "uB")
            eu1_t = load_u(w_d["eu1"], "uC")
            du0_t = load_u(w_d["du0"], "uD")
            dw1_t = load_u(w_d["dw1"], "uE")
            du1_t = load_u(w_d["du1"], "uF")
            wcomb_t = load_u(w_d["wcomb"], "uG")
            dw0_t = wp.tile([F, G3], BF, tag="dw0a", name="dw0a")
            nc.gpsimd.dma_start(dw0_t[:], dw0_d[:])
            w1t_t = wp.tile([128, KC], BF, tag="w1t", name="w1t")
            nc.gpsimd.dma_start(w1t_t[:], w1t_d[:])

            # hidden state ping-pong (bf16: matmul rhs + 2x DVE)
            h0b = [hp.tile([128, KC, BL], BF, tag=f"h0{i}", name=f"h0{i}") for i in range(2)]
            h1b = [hp.tile([128, KC, BL], BF, tag=f"h1{i}", name=f"h1{i}") for i in range(2)]

            def rzb(ct, g, m):
                c = ct * 8 + g * 4 + m
                return bias_t[:, c:c + 1]

            def bnb(ct, m):
                c = 40 + ct * 4 + m
                return bias_t[:, c:c + 1]

            def cnb(c4, m):
                c = 60 + c4 * 4 + m
                return bias_t[:, c:c + 1]

            def cell(gx_lhs, gx_rhs, gh_lhs, h_prev, h_out, ct, c4, gx_first,
                     first=False):
                """One GRU cell step, gate-major.

                gx_lhs/gx_rhs: matching lists of lhsT tiles ([*, G3]) and rhs APs
                gh_lhs: KC lhsT tiles for the recurrent projection
                h_prev/h_out: [128, KC, BL] bf16 state tiles
                ct/c4: bias column groups; gx_first: emit gx phase before gh
                first: h_prev is all zeros -- skip the gh/hn matmuls entirely

                PSUM bank packing (one pending accumulation group per 2KB bank):
                pa[m] = r | hn, pb[m] = z | xn. The r/z groups stay open across
                the two phases; hn/xn are single-phase groups, ordered so each
                bank's groups are strictly sequential.
                """
                pa = [pp.tile([128, 512], F32, tag=f"pA{m}", name=f"pA{m}") for m in range(4)]
                pb = [pp.tile([128, 512], F32, tag=f"pB{m}", name=f"pB{m}") for m in range(4)]
                rz = gp.tile([128, 4, 512], BF, tag="rz", name="rz")
                tt = gp.tile([128, 4, BL], BF, tag="g1", name="tt")
                vv = gp.tile([128, 4, BL], BF, tag="g2", name="vv")
                nn = gp.tile([128, 4, BL], BF, tag="gn", name="nn")
                hp_chunks = [h_prev[:, k, :] for k in range(KC)]

                def emit(lhs_list, rhs_list, m, goff, out_ap, opening, closing):
                    n = len(lhs_list)
                    for i, (lhs, rhs) in enumerate(zip(lhs_list, rhs_list, strict=True)):
                        nc.tensor.matmul(
                            out_ap, lhs[:, goff + m * 128:goff + (m + 1) * 128], rhs,
                            start=(opening and i == 0), stop=(closing and i == n - 1))

                def epilogue(m):
                    # r/z/xn/hn for tile m all closed: sigmoids (+bias) + n folds
                    nc.scalar.activation(rz[:, m, 0:BL], pa[m][:, 0:BL],
                                         AF.Sigmoid, bias=rzb(ct, 0, m))
                    nc.scalar.activation(rz[:, m, BL:2 * BL], pb[m][:, 0:BL],
                                         AF.Sigmoid, bias=rzb(ct, 1, m))
                    # tt = (hn + cn) * r ; vv = (xn + bn) + tt
                    nc.vector.scalar_tensor_tensor(
                        tt[:, m, :], pa[m][:, BL:2 * BL], cnb(c4, m),
                        rz[:, m, 0:BL], OP.add, OP.mult)
                    nc.vector.scalar_tensor_tensor(
                        vv[:, m, :], pb[m][:, BL:2 * BL], bnb(ct, m),
                        tt[:, m, :], OP.add, OP.add)

                if first:
                    for m in range(4):
                        emit(gx_lhs, gx_rhs, m, 1024, pb[m][:, BL:2 * BL], True, True)   # xn
                        emit(gx_lhs, gx_rhs, m, 0, pa[m][:, 0:BL], True, True)           # r
                        emit(gx_lhs, gx_rhs, m, 512, pb[m][:, 0:BL], True, True)         # z
                        # gh == 0: tt = r * cn ; vv = (xn + bn) + tt
                        nc.scalar.activation(rz[:, m, 0:BL], pa[m][:, 0:BL],
                                             AF.Sigmoid, bias=rzb(ct, 0, m))
                        nc.scalar.activation(rz[:, m, BL:2 * BL], pb[m][:, 0:BL],
                                             AF.Sigmoid, bias=rzb(ct, 1, m))
                        nc.vector.tensor_scalar(
                            tt[:, m, :], rz[:, m, 0:BL], cnb(c4, m), None, OP.mult)
                        nc.vector.scalar_tensor_tensor(
                            vv[:, m, :], pb[m][:, BL:2 * BL], bnb(ct, m),
                            tt[:, m, :], OP.add, OP.add)
                elif gx_first:
                    for m in range(4):
                        emit(gx_lhs, gx_rhs, m, 1024, pb[m][:, BL:2 * BL], True, True)   # xn
                        emit(gx_lhs, gx_rhs, m, 0, pa[m][:, 0:BL], True, False)          # r open
                        emit(gx_lhs, gx_rhs, m, 512, pb[m][:, 0:BL], True, False)        # z open
                    for m in range(4):
                        emit(gh_lhs, hp_chunks, m, 0, pa[m][:, 0:BL], False, True)       # r close
                        emit(gh_lhs, hp_chunks, m, 512, pb[m][:, 0:BL], False, True)     # z close
                        emit(gh_lhs, hp_chunks, m, 1024, pa[m][:, BL:2 * BL], True, True)  # hn
                        epilogue(m)
                else:
                    for m in range(4):
                        emit(gh_lhs, hp_chunks, m, 1024, pa[m][:, BL:2 * BL], True, True)  # hn
                        emit(gh_lhs, hp_chunks, m, 0, pa[m][:, 0:BL], True, False)       # r open
                        emit(gh_lhs, hp_chunks, m, 512, pb[m][:, 0:BL], True, False)     # z open
                    for m in range(4):
                        emit(gx_lhs, gx_rhs, m, 0, pa[m][:, 0:BL], False, True)          # r close
                        emit(gx_lhs, gx_rhs, m, 512, pb[m][:, 0:BL], False, True)        # z close
                        emit(gx_lhs, gx_rhs, m, 1024, pb[m][:, BL:2 * BL], True, True)   # xn
                        epilogue(m)

                nc.scalar.activation(nn[:, :, :], vv[:, :, :], AF.Tanh)
                # h' = n + z*(h - n)
                dd = gp.tile([128, 4, BL], BF, tag="g1", name="dd")
                ee = gp.tile([128, 4, BL], BF, tag="g2", name="ee")
                nc.vector.tensor_tensor(dd[:], h_prev[:, 0:KC, :], nn[:], OP.subtract)
                nc.vector.tensor_tensor(ee[:], rz[:, :, BL:2 * BL], dd[:], OP.mult)
                nc.vector.tensor_tensor(h_out[:, 0:KC, :], ee[:], nn[:], OP.add)

            for _rep in range(repeat):
                for i in range(2):
                    nc.vector.memzero(h0b[i][:])
                    nc.vector.memzero(h1b[i][:])

                # ---------------- encoder ----------------
                sc = None
                for t in range(lags):
                    if t % SRC_CHUNK == 0:
                        sc = sp.tile([F, SRC_CHUNK, BL], BF, tag="src", name=f"src{t}")
                        nc.sync.dma_start(sc[:], srcT_d[:, t:t + SRC_CHUNK, :])
                    j = t % SRC_CHUNK
                    p, q = t % 2, (t + 1) % 2
                    cell([ew0_t], [sc[:, j, :]], eu0_t, h0b[p], h0b[q],
                         CT_ENC0, C4_ENC0, gx_first=True, first=(t == 0))
                    cell(ew1_t, [h0b[q][:, k, :] for k in range(KC)], eu1_t,
                         h1b[p], h1b[q], CT_ENC1, C4_ENC1, gx_first=False,
                         first=(t == 0))
                sc_last = sc

                # ---------------- decoder ----------------
                for d in range(horizons):
                    p, q = (lags + d) % 2, (lags + d + 1) % 2
                    if d == 0:
                        cell([dw0_t], [sc_last[:, (lags - 1) % SRC_CHUNK, :]],
                             du0_t, h0b[p], h0b[q], CT_DEC0F, C4_DEC0, gx_first=True)
                    else:
                        cell(wcomb_t, [h1b[p][:, k, :] for k in range(KC)],
                             du0_t, h0b[p], h0b[q], CT_DEC0, C4_DEC0, gx_first=False)
                    cell(dw1_t, [h0b[q][:, k, :] for k in range(KC)], du1_t,
                         h1b[p], h1b[q], CT_DEC1, C4_DEC1, gx_first=False)
                    # out1[d] = W1 . h1_new   (b1 added on host). Lives in the
                    # pB3 bank, which the next cell touches last -- the pA0
                    # bank is the first one the next cell's matmuls need.
                    po = pp.tile([128, 512], F32, tag="pB3", name=f"po{d}")
                    for k in range(KC):
                        nc.tensor.matmul(po[0:1, 0:BL], w1t_t[:, k:k + 1],
                                         h1b[q][:, k, :], start=(k == 0), stop=(k == KC - 1))
                    osb = opool.tile([1, BL], F32, tag="o1", name=f"o{d}")
                    nc.scalar.copy(osb[:], po[0:1, 0:BL])
                    nc.sync.dma_start(out_d[d:d + 1, :], osb[:])

    nc.compile()
    return nc


def _host_prep(inputs):
    import ml_dtypes
    f32 = np.float32
    bf16 = ml_dtypes.bfloat16
    g = {k: np.asarray(v, dtype=f32) for k, v in inputs.items()
         if k not in ("train",)}
    src = g["src"]
    eW0, eU0, eb0, ec0 = g["eW0"], g["eU0"], g["eb0"], g["ec0"]
    eW1, eU1, eb1, ec1 = g["eW1"], g["eU1"], g["eb1"], g["ec1"]
    dW0, dU0, db0, dc0 = g["dW0"], g["dU0"], g["db0"], g["dc0"]
    dW1, dU1, db1, dc1 = g["dW1"], g["dU1"], g["db1"], g["dc1"]
    W1, b1, W4, b4 = g["W1"], g["b1"], g["W4"], g["b4"]

    Wcomb = (dW0 @ W4).astype(f32)                       # [1536, 512]
    dcomb = (db0 + dW0 @ b4).astype(f32)                 # [1536]

    biases = np.zeros((128, 76), f32)
    rz_sets = [(eb0 + ec0), (eb1 + ec1), (db0 + dc0), (dcomb + dc0), (db1 + dc1)]
    for ct, s in enumerate(rz_sets):
        for gate, goff in ((0, 0), (1, H)):
            for m in range(KC):
                biases[:, ct * 8 + gate * 4 + m] = s[goff + m * 128:goff + (m + 1) * 128]
    bn_sets = [eb0, eb1, db0, dcomb, db1]
    for ct, s in enumerate(bn_sets):
        sn = s[2 * H:]
        for m in range(KC):
            biases[:, 40 + ct * 4 + m] = sn[m * 128:(m + 1) * 128]
    cn_sets = [ec0, ec1, dc0, dc1]
    for c4, s in enumerate(cn_sets):
        sn = s[2 * H:]
        for m in range(KC):
            biases[:, 60 + c4 * 4 + m] = sn[m * 128:(m + 1) * 128]

    shared = {
        "eu0": eU0.T.astype(bf16), "ew1": eW1.T.astype(bf16),
        "eu1": eU1.T.astype(bf16),
        "du0": dU0.T.astype(bf16), "dw1": dW1.T.astype(bf16),
        "du1": dU1.T.astype(bf16),
        "wcomb": Wcomb.T.astype(bf16),
        "ew0": eW0.T.astype(bf16), "dw0": dW0.T.astype(bf16),
        "biases": biases,
        "w1t": W1[0].reshape(KC, 128).T.astype(bf16),
    }
    shared = {k: np.ascontiguousarray(v) for k, v in shared.items()}

    in_maps = []
    for c in range(NCORES):
        s = src[c * BL:(c + 1) * BL]                     # [256, 64, 64]
        sT = np.ascontiguousarray(s.transpose(2, 1, 0).astype(bf16))
        m = dict(shared)
        m["srcT"] = sT
        in_maps.append(m)
    return in_maps, float(b1[0])


class _Runner:
    """Build-once sharded PJRT runner (axon: 8 NeuronCores)."""

    def __init__(self, nc):
        import jax
        from jax.sharding import Mesh, PartitionSpec
        from jax.experimental.shard_map import shard_map
        from concourse import mybir
        from concourse.bass2jax import (_bass_exec_p, partition_id_tensor,
                                        install_neuronx_cc_hook)
        install_neuronx_cc_hook()
        self.jax = jax
        partition_name = nc.partition_id_tensor.name if nc.partition_id_tensor else None
        in_names, out_names, out_avals, zero_outs = [], [], [], []
        for alloc in nc.m.functions[0].allocations:
            if not isinstance(alloc, mybir.MemoryLocationSet):
                continue
            name = alloc.memorylocations[0].name
            if alloc.kind == "ExternalInput":
                if name != partition_name:
                    in_names.append(name)
            elif alloc.kind == "ExternalOutput":
                out_names.append(name)
                shape = tuple(alloc.tensor_shape)
                dtype = mybir.dt.np(alloc.dtype)
                out_avals.append(jax.core.ShapedArray(shape, dtype))
                zero_outs.append(np.zeros(shape, dtype))
        n_params = len(in_names)
        all_in = list(in_names) + list(out_names)
        if partition_name is not None:
            all_in.append(partition_name)
        self.in_names, self.out_names = in_names, out_names
        self.out_avals, self.zero_outs = out_avals, zero_outs

        def _body(*args):
            operands = list(args)
            if partition_name is not None:
                operands.append(partition_id_tensor())
            return tuple(_bass_exec_p.bind(
                *operands, out_avals=tuple(out_avals), in_names=tuple(all_in),
                out_names=tuple(out_names), lowering_input_output_aliases=(),
                sim_require_finite=True, sim_require_nnan=True, nc=nc))

        devices = jax.devices()[:NCORES]
        self.mesh = Mesh(np.asarray(devices), ("core",))
        in_specs = (PartitionSpec("core"),) * (n_params + len(out_names))
        out_specs = (PartitionSpec("core"),) * len(out_names)
        donate = tuple(range(n_params, n_params + len(out_names)))
        self.fn = jax.jit(
            shard_map(_body, mesh=self.mesh, in_specs=in_specs,
                      out_specs=out_specs, check_rep=False),
            donate_argnums=donate, keep_unused=True)
        self.sh = jax.sharding.NamedSharding(self.mesh, PartitionSpec("core"))

    def place(self, in_maps):
        n = NCORES
        self.placed = [
            self.jax.device_put(np.ascontiguousarray(
                np.concatenate([in_maps[c][nm] for c in range(n)], 0)), self.sh)
            for nm in self.in_names]

    def run(self):
        zeros = [self.jax.device_put(
            np.zeros((NCORES * z.shape[0], *z.shape[1:]), z.dtype), self.sh)
            for z in self.zero_outs]
        outs = self.fn(*self.placed, *zeros)
        self.jax.block_until_ready(outs)
        return outs

    def results(self, outs):
        return [
            {nm: np.asarray(outs[i]).reshape(NCORES, *self.out_avals[i].shape)[c]
             for i, nm in enumerate(self.out_names)}
            for c in range(NCORES)]


def get_runner(repeat=1):
    global _RUNNER
    key = ("r2", repeat)
    if _RUNNER is None or _RUNNER[0] != key:
        nc = _build_nc(repeat=repeat)
        _RUNNER = (key, _Runner(nc))
    return _RUNNER[1]


def kernel(**inputs) -> np.ndarray:
    in_maps, b1 = _host_prep(inputs)
    r = get_runner()
    r.place(in_maps)
    res = r.results(r.run())
    out = np.empty((B, HORIZONS), np.float32)
    for c in range(NCORES):
        out[c * BL:(c + 1) * BL] = res[c]["out"].T + b1
    return out


# revision 15
# speedup vs baseline: 45.0113x; 1.0016x over previous
"""GRU Seq2Seq Trainium2 kernel (nn_GRU_Seq2Seq_83219286327778).

Strategy: data-parallel over batch (2048 -> 8 x 256), gate-major transposed
layout on-device ([hidden/gate dim on partitions, batch on free dim]) so the
recurrence needs no transposes. Matmuls in bf16 (fp32 PSUM accumulate) to make
the per-matmul LDWEIGHTS cheap enough to hide under the moving stream; all
weights SBUF-resident from the start; biases folded into activation bias APs
and fused DVE ops (no rank-1 bias matmuls); gh emitted before gx in cells
whose input comes from freshly-computed state so the PE never stalls.
fc4 feedback folded into the next step's gx via Wcomb = dW0 @ W4.
"""
import sys
sys.path.insert(0, "/opt/trn_rl_repo")
import numpy as np

B, LAGS, HORIZONS, F, H = 2048, 64, 24, 64, 512
NCORES = 8
BL = B // NCORES           # 256 batch per core
G3 = 3 * H                 # 1536
KC = H // 128              # 4 k-chunks
SRC_CHUNK = 8              # timesteps per src DMA

# bias column layout in the [128, 76] biases tensor
# rz: ct*8 + g*4 + m   (ct in 0..4, g: 0=r 1=z, m tile)    cols 0..39
# bn: 40 + ct*4 + m    (x-side n bias per celltype)        cols 40..59
# cn: 60 + c4*4 + m    (h-side n bias per U-set)           cols 60..75
CT_ENC0, CT_ENC1, CT_DEC0F, CT_DEC0, CT_DEC1 = range(5)
C4_ENC0, C4_ENC1, C4_DEC0, C4_DEC1 = range(4)

_RUNNER = None


def _build_nc(repeat=1, lags=LAGS, horizons=HORIZONS):
    import concourse.tile as tile
    from concourse import mybir, bacc

    F32 = mybir.dt.float32
    BF = mybir.dt.bfloat16
    AF = mybir.ActivationFunctionType
    OP = mybir.AluOpType

    nc = bacc.Bacc("TRN2", target_bir_lowering=False)

    srcT_d = nc.dram_tensor("srcT", [F, LAGS, BL], BF, kind="ExternalInput")
    wnames = ["eu0", "ew1", "eu1", "du0", "dw1", "du1", "wcomb"]
    w_d = {n: nc.dram_tensor(n, [H, G3], BF, kind="ExternalInput") for n in wnames}
    ew0_d = nc.dram_tensor("ew0", [F, G3], BF, kind="ExternalInput")
    dw0_d = nc.dram_tensor("dw0", [F, G3], BF, kind="ExternalInput")
    bias_d = nc.dram_tensor("biases", [128, 76], F32, kind="ExternalInput")
    w1t_d = nc.dram_tensor("w1t", [128, KC], BF, kind="ExternalInput")
    out_d = nc.dram_tensor("out", [HORIZONS, BL], F32, kind="ExternalOutput")

    with tile.TileContext(nc) as tc:
        with tc.tile_pool(name="wp", bufs=1) as wp, \
             tc.tile_pool(name="sp", bufs=2) as sp, \
             tc.tile_pool(name="hp", bufs=1) as hp, \
             tc.tile_pool(name="gp", bufs=3) as gp, \
             tc.tile_pool(name="op_", bufs=2) as opool, \
             tc.tile_pool(name="pp", bufs=1, space="PSUM") as pp:

            # ---- persistent small tensors ----
            # DMA issue order = queue order: first-needed tensors first so the
            # first cell isn't stuck behind the 10MB weight stream. src chunks
            # ride the (idle) sync queue, concurrent with the gpsimd stream.
            bias_t = wp.tile([128, 76], F32, tag="bias", name="bias")
            nc.sync.dma_start(bias_t[:], bias_d[:])
            ew0_t = wp.tile([F, G3], BF, tag="w0a", name="w0a")
            nc.gpsimd.dma_start(ew0_t[:], ew0_d[:])
            # dummy activation: pulls the sigmoid/tanh ACT table load (~2.7us)
            # into the weight-DMA window instead of the first cell's epilogue
            dummy_t = wp.tile([1, 1], F32, tag="dummy", name="dummy")
            nc.scalar.activation(dummy_t[:], bias_t[0:1, 0:1],
                                 AF.Sigmoid)

            def load_u(dram, tagbase):
                ts_ = []
                for k in range(KC):
                    t = wp.tile([128, G3], BF, tag=f"{tagbase}{k}", name=f"{tagbase}{k}")
                    nc.gpsimd.dma_start(t[:], dram[k * 128:(k + 1) * 128, :])
                    ts_.append(t)
                return ts_

            # all weights resident for the whole kernel, in first-use order
            eu0_t = load_u(w_d["eu0"], "uA")
            ew1_t = load_u(w_d["ew1"], "uB")
            eu1_t = load_u(w_d["eu1"], "uC")
            du0_t = load_u(w_d["du0"], "uD")
            dw1_t = load_u(w_d["dw1"], "uE")
            du1_t = load_u(w_d["du1"], "uF")
            wcomb_t = load_u(w_d["wcomb"], "uG")
            dw0_t = wp.tile([F, G3], BF, tag="dw0a", name="dw0a")
            nc.gpsimd.dma_start(dw0_t[:], dw0_d[:])
            w1t_t = wp.tile([128, KC], BF, tag="w1t", name="w1t")
            nc.gpsimd.dma_start(w1t_t[:], w1t_d[:])

            # hidden state ping-pong (bf16: matmul rhs + 2x DVE)
            h0b = [hp.tile([128, KC, BL], BF, tag=f"h0{i}", name=f"h0{i}") for i in range(2)]
            h1b = [hp.tile([128, KC, BL], BF, tag=f"h1{i}", name=f"h1{i}") for i in range(2)]

            def rzb(ct, g, m):
                c = ct * 8 + g * 4 + m
                return bias_t[:, c:c + 1]

            def bnb(ct, m):
                c = 40 + ct * 4 + m
                return bias_t[:, c:c + 1]

            def cnb(c4, m):
                c = 60 + c4 * 4 + m
                return bias_t[:, c:c + 1]

            def cell(gx_lhs, gx_rhs, gh_lhs, h_prev, h_out, ct, c4, gx_first,
                     first=False):
                """One GRU cell step, gate-major.

                gx_lhs/gx_rhs: matching lists of lhsT tiles ([*, G3]) and rhs APs
                gh_lhs: KC lhsT tiles for the recurrent projection
                h_prev/h_out: [128, KC, BL] bf16 state tiles
                ct/c4: bias column groups; gx_first: emit gx phase before gh
                first: h_prev is all zeros -- skip the gh/hn matmuls entirely

                PSUM bank packing (one pending accumulation group per 2KB bank):
                pa[m] = r | hn, pb[m] = z | xn. The r/z groups stay open across
                the two phases; hn/xn are single-phase groups, ordered so each
                bank's groups are strictly sequential.
                """
                pa = [pp.tile([128, 512], F32, tag=f"pA{m}", name=f"pA{m}") for m in range(4)]
                pb = [pp.tile([128, 512], F32, tag=f"pB{m}", name=f"pB{m}") for m in range(4)]
                rz = gp.tile([128, 4, 512], BF, tag="rz", name="rz")
                tt = gp.tile([128, 4, BL], BF, tag="g1", name="tt")
                vv = gp.tile([128, 4, BL], BF, tag="g2", name="vv")
                nn = gp.tile([128, 4, BL], BF, tag="gn", name="nn")
                hp_chunks = [h_prev[:, k, :] for k in range(KC)]

                def emit(lhs_list, rhs_list, m, goff, out_ap, opening, closing):
                    n = len(lhs_list)
                    for i, (lhs, rhs) in enumerate(zip(lhs_list, rhs_list, strict=True)):
                        nc.tensor.matmul(
                            out_ap, lhs[:, goff + m * 128:goff + (m + 1) * 128], rhs,
                            start=(opening and i == 0), stop=(closing and i == n - 1))

                def epilogue(m):
                    # r/z/xn/hn for tile m all closed: sigmoids (+bias) + n folds
                    nc.scalar.activation(rz[:, m, 0:BL], pa[m][:, 0:BL],
                                         AF.Sigmoid, bias=rzb(ct, 0, m))
                    nc.scalar.activation(rz[:, m, BL:2 * BL], pb[m][:, 0:BL],
                                         AF.Sigmoid, bias=rzb(ct, 1, m))
                    # tt = (hn + cn) * r ; vv = (xn + bn) + tt
                    nc.vector.scalar_tensor_tensor(
                        tt[:, m, :], pa[m][:, BL:2 * BL], cnb(c4, m),
                        rz[:, m, 0:BL], OP.add, OP.mult)
                    nc.vector.scalar_tensor_tensor(
                        vv[:, m, :], pb[m][:, BL:2 * BL], bnb(ct, m),
                        tt[:, m, :], OP.add, OP.add)

                if first:
                    for m in range(4):
                        emit(gx_lhs, gx_rhs, m, 1024, pb[m][:, BL:2 * BL], True, True)   # xn
                        emit(gx_lhs, gx_rhs, m, 0, pa[m][:, 0:BL], True, True)           # r
                        emit(gx_lhs, gx_rhs, m, 512, pb[m][:, 0:BL], True, True)         # z
                        # gh == 0: tt = r * cn ; vv = (xn + bn) + tt
                        nc.scalar.activation(rz[:, m, 0:BL], pa[m][:, 0:BL],
                                             AF.Sigmoid, bias=rzb(ct, 0, m))
                        nc.scalar.activation(rz[:, m, BL:2 * BL], pb[m][:, 0:BL],
                                             AF.Sigmoid, bias=rzb(ct, 1, m))
                        nc.vector.tensor_scalar(
                            tt[:, m, :], rz[:, m, 0:BL], cnb(c4, m), None, OP.mult)
                        nc.vector.scalar_tensor_tensor(
                            vv[:, m, :], pb[m][:, BL:2 * BL], bnb(ct, m),
                            tt[:, m, :], OP.add, OP.add)
                elif gx_first:
                    for m in range(4):
                        emit(gx_lhs, gx_rhs, m, 1024, pb[m][:, BL:2 * BL], True, True)   # xn
                        emit(gx_lhs, gx_rhs, m, 0, pa[m][:, 0:BL], True, False)          # r open
                        emit(gx_lhs, gx_rhs, m, 512, pb[m][:, 0:BL], True, False)        # z open
                    for m in range(4):
                        emit(gh_lhs, hp_chunks, m, 0, pa[m][:, 0:BL], False, True)       # r close
                        emit(gh_lhs, hp_chunks, m, 512, pb[m][:, 0:BL], False, True)     # z close
                        emit(gh_lhs, hp_chunks, m, 1024, pa[m][:, BL:2 * BL], True, True)  # hn
                        epilogue(m)
                else:
                    for m in range(4):
                        emit(gh_lhs, hp_chunks, m, 1024, pa[m][:, BL:2 * BL], True, True)  # hn
                        emit(gh_lhs, hp_chunks, m, 0, pa[m][:, 0:BL], True, False)       # r open
                        emit(gh_lhs, hp_chunks, m, 512, pb[m][:, 0:BL], True, False)     # z open
                    for m in range(4):
                        emit(gx_lhs, gx_rhs, m, 0, pa[m][:, 0:BL], False, True)          # r close
                        emit(gx_lhs, gx_rhs, m, 512, pb[m][:, 0:BL], False, True)        # z close
                        emit(gx_lhs, gx_rhs, m, 1024, pb[m][:, BL:2 * BL], True, True)   # xn
                        epilogue(m)

                nc.scalar.activation(nn[:, :, :], vv[:, :, :], AF.Tanh)
                # h' = n + z*(h - n)
                dd = gp.tile([128, 4, BL], BF, tag="g1", name="dd")
                ee = gp.tile([128, 4, BL], BF, tag="g2", name="ee")
                nc.vector.tensor_tensor(dd[:], h_prev[:, 0:KC, :], nn[:], OP.subtract)
                nc.vector.tensor_tensor(ee[:], rz[:, :, BL:2 * BL], dd[:], OP.mult)
                nc.vector.tensor_tensor(h_out[:, 0:KC, :], ee[:], nn[:], OP.add)

            for _rep in range(repeat):
                for i in range(2):
                    nc.vector.memzero(h0b[i][:])
                    nc.vector.memzero(h1b[i][:])

                # ---------------- encoder ----------------
                sc = None
                for t in range(lags):
                    if t % SRC_CHUNK == 0:
                        sc = sp.tile([F, SRC_CHUNK, BL], BF, tag="src", name=f"src{t}")
                        nc.sync.dma_start(sc[:], srcT_d[:, t:t + SRC_CHUNK, :])
                    j = t % SRC_CHUNK
                    p, q = t % 2, (t + 1) % 2
                    cell([ew0_t], [sc[:, j, :]], eu0_t, h0b[p], h0b[q],
                         CT_ENC0, C4_ENC0, gx_first=True, first=(t == 0))
                    cell(ew1_t, [h0b[q][:, k, :] for k in range(KC)], eu1_t,
                         h1b[p], h1b[q], CT_ENC1, C4_ENC1, gx_first=False,
                         first=(t == 0))
                sc_last = sc

                # ---------------- decoder ----------------
                for d in range(horizons):
                    p, q = (lags + d) % 2, (lags + d + 1) % 2
                    if d == 0:
                        cell([dw0_t], [sc_last[:, (lags - 1) % SRC_CHUNK, :]],
                             du0_t, h0b[p], h0b[q], CT_DEC0F, C4_DEC0, gx_first=True)
                    else:
                        cell(wcomb_t, [h1b[p][:, k, :] for k in range(KC)],
                             du0_t, h0b[p], h0b[q], CT_DEC0, C4_DEC0, gx_first=False)
                    cell(dw1_t, [h0b[q][:, k, :] for k in range(KC)], du1_t,
                         h1b[p], h1b[q], CT_DEC1, C4_DEC1, gx_first=False)
                    # out1[d] = W1 . h1_new   (b1 added on host). Lives in the
                    # pB3 bank, which the next cell touches last -- the pA0
                    # bank is the first one the next cell's matmuls need.
                    po = pp.tile([128, 512], F32, tag="pB3", name=f"po{d}")
                    for k in range(KC):
                        nc.tensor.matmul(po[0:1, 0:BL], w1t_t[:, k:k + 1],
                                         h1b[q][:, k, :], start=(k == 0), stop=(k == KC - 1))
                    osb = opool.tile([1, BL], F32, tag="o1", name=f"o{d}")
                    nc.scalar.copy(osb[:], po[0:1, 0:BL])
                    nc.sync.dma_start(out_d[d:d + 1, :], osb[:])

    nc.compile()
    return nc


def _host_prep(inputs):
    import ml_dtypes
    f32 = np.float32
    bf16 = ml_dtypes.bfloat16
    g = {k: np.asarray(v, dtype=f32) for k, v in inputs.items()
         if k not in ("train",)}
    src = g["src"]
    eW0, eU0, eb0, ec0 = g["eW0"], g["eU0"], g["eb0"], g["ec0"]
    eW1, eU1, eb1, ec1 = g["eW1"], g["eU1"], g["eb1"], g["ec1"]
    dW0, dU0, db0, dc0 = g["dW0"], g["dU0"], g["db0"], g["dc0"]
    dW1, dU1, db1, dc1 = g["dW1"], g["dU1"], g["db1"], g["dc1"]
    W1, b1, W4, b4 = g["W1"], g["b1"], g["W4"], g["b4"]

    Wcomb = (dW0 @ W4).astype(f32)                       # [1536, 512]
    dcomb = (db0 + dW0 @ b4).astype(f32)                 # [1536]

    biases = np.zeros((128, 76), f32)
    rz_sets = [(eb0 + ec0), (eb1 + ec1), (db0 + dc0), (dcomb + dc0), (db1 + dc1)]
    for ct, s in enumerate(rz_sets):
        for gate, goff in ((0, 0), (1, H)):
            for m in range(KC):
                biases[:, ct * 8 + gate * 4 + m] = s[goff + m * 128:goff + (m + 1) * 128]
    bn_sets = [eb0, eb1, db0, dcomb, db1]
    for ct, s in enumerate(bn_sets):
        sn = s[2 * H:]
        for m in range(KC):
            biases[:, 40 + ct * 4 + m] = sn[m * 128:(m + 1) * 128]
    cn_sets = [ec0, ec1, dc0, dc1]
    for c4, s in enumerate(cn_sets):
        sn = s[2 * H:]
        for m in range(KC):
            biases[:, 60 + c4 * 4 + m] = sn[m * 128:(m + 1) * 128]

    shared = {
        "eu0": eU0.T.astype(bf16), "ew1": eW1.T.astype(bf16),
        "eu1": eU1.T.astype(bf16),
        "du0": dU0.T.astype(bf16), "dw1": dW1.T.astype(bf16),
        "du1": dU1.T.astype(bf16),
        "wcomb": Wcomb.T.astype(bf16),
        "ew0": eW0.T.astype(bf16), "dw0": dW0.T.astype(bf16),
        "biases": biases,
        "w1t": W1[0].reshape(KC, 128).T.astype(bf16),
    }
    shared = {k: np.ascontiguousarray(v) for k, v in shared.items()}

    in_maps = []
    for c in range(NCORES):
        s = src[c * BL:(c + 1) * BL]                     # [256, 64, 64]
        sT = np.ascontiguousarray(s.transpose(2, 1, 0).astype(bf16))
        m = dict(shared)
        m["srcT"] = sT
        in_maps.append(m)
    return in_maps, float(b1[0])


class _Runner:
    """Build-once sharded PJRT runner (axon: 8 NeuronCores)."""

    def __init__(self, nc):
        import jax
        from jax.sharding import Mesh, PartitionSpec
        from jax.experimental.shard_map import shard_map
        from concourse import mybir
        from concourse.bass2jax import (_bass_exec_p, partition_id_tensor,
                                        install_neuronx_cc_hook)
        install_neuronx_cc_hook()
        self.jax = jax
        partition_name = nc.partition_id_tensor.name if nc.partition_id_tensor else None
        in_names, out_names, out_avals, zero_outs = [], [], [], []
        for alloc in nc.m.functions[0].allocations:
            if not isinstance(alloc, mybir.MemoryLocationSet):
                continue
            name = alloc.memorylocations[0].name
            if alloc.kind == "ExternalInput":
                if name != partition_name:
                    in_names.append(name)
            elif alloc.kind == "ExternalOutput":
                out_names.append(name)
                shape = tuple(alloc.tensor_shape)
                dtype = mybir.dt.np(alloc.dtype)
                out_avals.append(jax.core.ShapedArray(shape, dtype))
                zero_outs.append(np.zeros(shape, dtype))
        n_params = len(in_names)
        all_in = list(in_names) + list(out_names)
        if partition_name is not None:
            all_in.append(partition_name)
        self.in_names, self.out_names = in_names, out_names
        self.out_avals, self.zero_outs = out_avals, zero_outs

        def _body(*args):
            operands = list(args)
            if partition_name is not None:
                operands.append(partition_id_tensor())
            return tuple(_bass_exec_p.bind(
                *operands, out_avals=tuple(out_avals), in_names=tuple(all_in),
                out_names=tuple(out_names), lowering_input_output_aliases=(),
                sim_require_finite=True, sim_require_nnan=True, nc=nc))

        devices = jax.devices()[:NCORES]
        self.mesh = Mesh(np.asarray(devices), ("core",))
        in_specs = (PartitionSpec("core"),) * (n_params + len(out_names))
        out_specs = (PartitionSpec("core"),) * len(out_names)
        donate = tuple(range(n_params, n_params + len(out_names)))
        self.fn = jax.jit(
            shard_map(_body, mesh=self.mesh, in_specs=in_specs,
                      out_specs=out_specs, check_rep=False),
            donate_argnums=donate, keep_unused=True)
        self.sh = jax.sharding.NamedSharding(self.mesh, PartitionSpec("core"))

    def place(self, in_maps):
        n = NCORES
        self.placed = [
            self.jax.device_put(np.ascontiguousarray(
                np.concatenate([in_maps[c][nm] for c in range(n)], 0)), self.sh)
            for nm in self.in_names]

    def run(self):
        zeros = [self.jax.device_put(
            np.zeros((NCORES * z.shape[0], *z.shape[1:]), z.dtype), self.sh)
            for z in self.zero_outs]
        outs = self.fn(*self.placed, *zeros)
        self.jax.block_until_ready(outs)
        return outs

    def results(self, outs):
        return [
            {nm: np.asarray(outs[i]).reshape(NCORES, *self.out_avals[i].shape)[c]
             for i, nm in enumerate(self.out_names)}
            for c in range(NCORES)]


def get_runner(repeat=1):
    global _RUNNER
    key = ("r2", repeat)
    if _RUNNER is None or _RUNNER[0] != key:
        nc = _build_nc(repeat=repeat)
        _RUNNER = (key, _Runner(nc))
    return _RUNNER[1]


def kernel(**inputs) -> np.ndarray:
    in_maps, b1 = _host_prep(inputs)
    r = get_runner()
    r.place(in_maps)
    res = r.results(r.run())
    out = np.empty((B, HORIZONS), np.float32)
    for c in range(NCORES):
        out[c * BL:(c + 1) * BL] = res[c]["out"].T + b1
    return out
